# revision 19
# baseline (speedup 1.0000x reference)
"""MLA (multi-head latent attention) forward on 8 Trainium2 NeuronCores.

Sharding: token-sharded compress (low-rank latents) + AllGather of latents +
head-sharded attention (2 heads/core) + head-sliced out-projection partials
summed on host.

All device matmuls run in bf16 with fp32 PSUM accumulation. Activations are
held transposed ([feature, token]) so every matmul contracts along partitions
with zero on-device transposes of activations (probs transposes for attn@V go
through the PE).

Self-contained: hardcodes the problem shapes from the reference
(B=2, T=2048, D=2048, H=16, ND=128, RD=64, VD=128, QR=1536, KVR=512).
"""
import math
import sys
import types

import numpy as np
import ml_dtypes

BF16 = ml_dtypes.bfloat16

# problem shapes
B, T, D = 2, 2048, 2048
H, ND, RD, VD = 16, 128, 64, 128
QR, KVR = 1536, 512
EPS = 1e-6
N_CORES = 8
HPC = H // N_CORES            # heads per core = 2
R = B * T                     # 4096 global rows
RPC = R // N_CORES            # 512 rows per core slab
NQT = T // 128                # 16 q-tiles per batch
SCALE = 1.0 / math.sqrt(ND + RD)
NEG = -30000.0

# latent chunk layout: 12 q chunks, 4 kv chunks, 1 krope chunk (64 rows used)
QCH = QR // 128               # 12
KCH = KVR // 128              # 4
LCH = QCH + KCH + 1           # 17


def _register_ntff_hook():
    try:
        import antenv
    except ImportError:
        return
    if "antenv.axon_hooks" in sys.modules:
        return
    mod = types.ModuleType("antenv.axon_hooks")
    mod._hook = None
    mod.set_axon_ntff_profile_hook = lambda h: setattr(mod, "_hook", h)
    mod.get_axon_ntff_profile_hook = lambda: mod._hook
    sys.modules["antenv.axon_hooks"] = mod
    antenv.axon_hooks = mod
    try:
        from trn_agent_boot.trn_boot import _ntff_profile_via_ctypes
        mod.set_axon_ntff_profile_hook(
            _ntff_profile_via_ctypes("/opt/axon/libaxon_pjrt.so"))
    except Exception:
        pass


def _deint(n):
    """deinterleave permutation for rope dims: [0,2,...,n-2, 1,3,...,n-1]"""
    return np.concatenate([np.arange(0, n, 2), np.arange(1, n, 2)])


def host_prepare(x, w_cq, g_q, w_ckv, g_kv, w_dq_nope, w_dq_rope,
                 w_dk_nope, w_dv, w_k_rope, w_out):
    """Build per-core input maps (numpy, bf16 for matmul operands)."""
    x = np.asarray(x, np.float32)
    xf = x.reshape(R, D)
    perm64 = _deint(RD)

    # fold rmsnorm gains into decompress weights
    wdqn_f = np.asarray(w_dq_nope, np.float32) * np.asarray(g_q, np.float32)[:, None]
    wdqr_f = np.asarray(w_dq_rope, np.float32) * np.asarray(g_q, np.float32)[:, None]
    wdkn_f = np.asarray(w_dk_nope, np.float32) * np.asarray(g_kv, np.float32)[:, None]
    wdv_f = np.asarray(w_dv, np.float32) * np.asarray(g_kv, np.float32)[:, None]

    wcq = np.asarray(w_cq, np.float32).astype(BF16)
    wckv = np.asarray(w_ckv, np.float32).astype(BF16)
    wkr = np.asarray(w_k_rope, np.float32)[:, perm64].astype(BF16)

    # rope tables, deinterleaved freq order, indexed by global row (b*T + t)
    inv_freq = 1.0 / (10000.0 ** (np.arange(0, RD, 2, dtype=np.float32) / RD))  # [32]
    t_of_row = np.tile(np.arange(T, dtype=np.float32), B)                       # [R]
    ang = inv_freq[:, None] * t_of_row[None, :]                                 # [32, R]
    cos32 = np.cos(ang).astype(np.float32)
    sin32 = np.sin(ang).astype(np.float32)

    # transposed causal masks for 512-wide q-groups: for relative key chunk m
    # (0..3), maskT[m][kr, qc] = 0 if (m*128 + kr) <= qc else NEG
    kr = np.arange(128)[:, None]
    qc = np.arange(512)[None, :]
    masks4 = np.stack([np.where(m * 128 + kr <= qc, 0.0, NEG)
                       for m in range(4)]).astype(np.float32)

    in_maps = []
    for c in range(N_CORES):
        h0, h1 = 2 * c, 2 * c + 1
        # w_dq_rope columns for head h: [h*RD, (h+1)*RD); per-head [even32, odd32]
        qr_cols = np.concatenate([h0 * RD + perm64, h1 * RD + perm64])
        nope_cols = np.concatenate(
            [np.arange(h0 * ND, (h0 + 1) * ND), np.arange(h1 * ND, (h1 + 1) * ND)])
        v_cols = np.concatenate(
            [np.arange(h0 * VD, (h0 + 1) * VD), np.arange(h1 * VD, (h1 + 1) * VD)])
        in_maps.append({
            "xt": np.ascontiguousarray(xf[c * RPC:(c + 1) * RPC].T).astype(BF16),
            "wcq": wcq,
            "wckv": wckv,
            "wkr": wkr,
            "wdqn": wdqn_f[:, nope_cols].astype(BF16),
            "wdqr": wdqr_f[:, qr_cols].astype(BF16),
            "wdkn": wdkn_f[:, nope_cols].astype(BF16),
            "wdv": wdv_f[:, v_cols].astype(BF16),
            "wout": np.ascontiguousarray(
                np.asarray(w_out, np.float32)[v_cols, :]).astype(BF16),
            "cos32": cos32,
            "sin32": sin32,
            "masks4": masks4,
        })
    return in_maps


def build_program():
    import concourse.bass as bass
    import concourse.tile as tile
    from concourse import bacc, mybir

    dt = mybir.dt
    F32, BF = dt.float32, dt.bfloat16
    AF = mybir.ActivationFunctionType

    nc = bacc.Bacc(None, target_bir_lowering=False)

    # ---- I/O ----
    xt = nc.declare_dram_parameter("xt", [D, RPC], BF, isOutput=False)
    wcq = nc.declare_dram_parameter("wcq", [D, QR], BF, isOutput=False)
    wckv = nc.declare_dram_parameter("wckv", [D, KVR], BF, isOutput=False)
    wkr = nc.declare_dram_parameter("wkr", [D, RD], BF, isOutput=False)
    wdqn = nc.declare_dram_parameter("wdqn", [QR, 2 * ND], BF, isOutput=False)
    wdqr = nc.declare_dram_parameter("wdqr", [QR, 2 * RD], BF, isOutput=False)
    wdkn = nc.declare_dram_parameter("wdkn", [KVR, 2 * ND], BF, isOutput=False)
    wdv = nc.declare_dram_parameter("wdv", [KVR, 2 * VD], BF, isOutput=False)
    wout = nc.declare_dram_parameter("wout", [2 * VD, D], BF, isOutput=False)
    cos32 = nc.declare_dram_parameter("cos32", [RD // 2, R], F32, isOutput=False)
    sin32 = nc.declare_dram_parameter("sin32", [RD // 2, R], F32, isOutput=False)
    masks4 = nc.declare_dram_parameter("masks4", [4, 128, 512], F32, isOutput=False)

    kno = nc.declare_dram_parameter("kno", [R, 2 * ND], F32, isOutput=True)
    vo = nc.declare_dram_parameter("vo", [R, 2 * VD], F32, isOutput=True)
    kro = nc.declare_dram_parameter("kro", [RD, RPC], F32, isOutput=True)
    outp = nc.declare_dram_parameter("outp", [R, D], F32, isOutput=True)

    lat_kv_loc = nc.dram_tensor("lat_kv_loc", [KCH + 1, 128, RPC], BF)
    lat_q_loc = nc.dram_tensor("lat_q_loc", [QCH, 128, RPC], BF)
    lat_kv_g = nc.dram_tensor("lat_kv_g", [N_CORES, KCH + 1, 128, RPC], BF,
                              addr_space="Shared")
    lat_q_g = nc.dram_tensor("lat_q_g", [N_CORES, QCH, 128, RPC], BF,
                             addr_space="Shared")

    with tile.TileContext(nc) as tc:
        # ================= phase 1: compress own slab =================
        with tc.tile_pool(name="p1w", bufs=1) as p1w, \
             tc.tile_pool(name="p1s", bufs=1) as p1s, \
             tc.tile_pool(name="p1t", bufs=3) as p1t, \
             tc.tile_pool(name="p1p", bufs=2, space="PSUM") as p1p, \
             tc.tile_pool(name="p1pk", bufs=1, space="PSUM") as p1pk, \
             tc.tile_pool(name="p1q", bufs=1, space="PSUM") as p1q:
            xt_sb = p1w.tile([128, D // 128, RPC], BF, tag="xt")
            nc.sync.dma_start(out=xt_sb,
                              in_=xt[:, :].rearrange("(kc p) r -> p kc r", p=128))
            wcq_sb = p1w.tile([128, D // 128, QR], BF, tag="wcq")
            nc.sync.dma_start(out=wcq_sb,
                              in_=wcq[:, :].rearrange("(kc p) m -> p kc m", p=128))
            wckv_sb = p1w.tile([128, D // 128, KVR], BF, tag="wckv")
            nc.sync.dma_start(out=wckv_sb,
                              in_=wckv[:, :].rearrange("(kc p) m -> p kc m", p=128))
            wkr_sb = p1w.tile([128, D // 128, RD], BF, tag="wkr")
            nc.sync.dma_start(out=wkr_sb,
                              in_=wkr[:, :].rearrange("(kc p) m -> p kc m", p=128))
            ones128 = p1w.tile([128, 1], BF, tag="ones128")
            nc.vector.memset(ones128, 1.0)
            ones_k1 = p1w.tile([1, 128], F32, tag="ones_k1")
            nc.vector.memset(ones_k1, 1.0)
            eps_sb = p1w.tile([1, 1], F32, tag="eps_sb")
            nc.vector.memset(eps_sb, EPS)

            qstg = p1s.tile([128, QCH, RPC], F32, tag="qstg")
            kstg = p1s.tile([128, KCH, RPC], F32, tag="kstg")
            latq_sb = p1s.tile([128, QCH, RPC], BF, tag="latq_sb")
            latkv_sb = p1s.tile([128, KCH + 1, RPC], BF, tag="latkv_sb")

            psq_q = p1q.tile([1, RPC], F32, tag="psq_q")
            psq_k = p1q.tile([1, RPC], F32, tag="psq_k")

            def compress(n_ch, w_sb, stg, psq):
                for j in range(n_ch):
                    pc = p1p.tile([128, RPC], F32, tag="pc")
                    for kc in range(D // 128):
                        nc.tensor.matmul(pc, w_sb[:, kc, j * 128:(j + 1) * 128],
                                         xt_sb[:, kc, :],
                                         start=(kc == 0), stop=(kc == D // 128 - 1))
                    nc.scalar.activation(out=stg[:, j, :], in_=pc, func=AF.Copy)
                    sq = p1t.tile([128, RPC], BF, tag="sq")
                    nc.scalar.activation(out=sq, in_=pc, func=AF.Square)
                    nc.tensor.matmul(psq, ones128, sq,
                                     start=(j == 0), stop=(j == n_ch - 1),
                                     skip_group_check=True)

            def normalize(stg, psq, n_ch, dim, dst_sb, dst_off):
                tsd = p1t.tile([1, RPC], F32, tag="tsd")
                nc.scalar.activation(out=tsd, in_=psq, func=AF.Sqrt,
                                     scale=1.0 / dim, bias=eps_sb)
                rstd = p1t.tile([1, RPC], F32, tag="rstd")
                nc.vector.reciprocal(rstd, tsd)
                pbc = p1pk.tile([128, RPC], F32, tag="pbc")
                nc.tensor.matmul(pbc, ones_k1, rstd, start=True, stop=True)
                bcast = p1t.tile([128, RPC], F32, tag="bcast")
                nc.scalar.activation(out=bcast, in_=pbc, func=AF.Copy)
                for j in range(n_ch):
                    nc.vector.tensor_mul(dst_sb[:, dst_off + j, :],
                                         stg[:, j, :], bcast)

            # ---- kv latents + k_rope first, gather early ----
            compress(KCH, wckv_sb, kstg, psq_k)
            pkr = p1pk.tile([64, RPC], F32, tag="pkr")
            for kc in range(D // 128):
                nc.tensor.matmul(pkr, wkr_sb[:, kc, :], xt_sb[:, kc, :],
                                 start=(kc == 0), stop=(kc == D // 128 - 1))
            krstg = p1t.tile([64, RPC], F32, tag="krstg")
            nc.scalar.activation(out=krstg, in_=pkr, func=AF.Copy)
            nc.sync.dma_start(out=kro[:, :], in_=krstg)
            nc.vector.tensor_copy(out=latkv_sb[0:64, KCH, :], in_=krstg)
            nc.vector.memset(latkv_sb[64:128, KCH, :], 0.0)
            normalize(kstg, psq_k, KCH, KVR, latkv_sb, 0)
            nc.sync.dma_start(
                out=lat_kv_loc[:, :, :].rearrange("c p r -> p c r"), in_=latkv_sb)
            with tc.tile_critical():
                with nc.semaphore() as csem1:
                    nc.gpsimd.collective_compute(
                        "AllGather", mybir.AluOpType.bypass,
                        replica_groups=[list(range(N_CORES))],
                        ins=[lat_kv_loc[:]], outs=[lat_kv_g[:]],
                    ).then_inc(csem1, 1)
                    nc.gpsimd.wait_ge(csem1, 1)

            # ---- q latents, gather second ----
            compress(QCH, wcq_sb, qstg, psq_q)
            normalize(qstg, psq_q, QCH, QR, latq_sb, 0)
            nc.sync.dma_start(
                out=lat_q_loc[:, :, :].rearrange("c p r -> p c r"), in_=latq_sb)
            with tc.tile_critical():
                with nc.semaphore() as csem2:
                    nc.gpsimd.collective_compute(
                        "AllGather", mybir.AluOpType.bypass,
                        replica_groups=[list(range(N_CORES))],
                        ins=[lat_q_loc[:]], outs=[lat_q_g[:]],
                    ).then_inc(csem2, 1)
                    nc.gpsimd.wait_ge(csem2, 1)

        # ============ phase 2: decompress q/k/v for own heads ============
        caches_cm = tc.tile_pool(name="caches", bufs=1)
        caches = caches_cm.__enter__()
        qn_cache = caches.tile([128, HPC, R], BF, tag="qn_cache")
        qrc0 = caches.tile([64, R], BF, tag="qrc0")
        qrc1 = caches.tile([64, R], BF, tag="qrc1")
        qrc = [qrc0, qrc1]
        kn_cache = caches.tile([128, HPC, R], BF, tag="kn_cache")
        kr_cache = caches.tile([64, R], BF, tag="kr_cache")
        v_cache = caches.tile([128, R // 128, 2 * VD], BF, tag="v_cache")
        cs_sb = caches.tile([32, R], F32, tag="cs_sb")
        sn_sb = caches.tile([32, R], F32, tag="sn_sb")
        nc.sync.dma_start(out=cs_sb, in_=cos32[:, :])
        nc.sync.dma_start(out=sn_sb, in_=sin32[:, :])

        with tc.tile_pool(name="p2w", bufs=1) as p2w, \
             tc.tile_pool(name="p2l", bufs=2) as p2l, \
             tc.tile_pool(name="p2t", bufs=4) as p2t, \
             tc.tile_pool(name="p2p", bufs=4, space="PSUM") as p2p:
            wdqn_sb = p2w.tile([128, QCH, 2 * ND], BF, tag="wdqn")
            nc.sync.dma_start(out=wdqn_sb,
                              in_=wdqn[:, :].rearrange("(kc p) m -> p kc m", p=128))
            wdqr_sb = p2w.tile([128, QCH, 2 * RD], BF, tag="wdqr")
            nc.sync.dma_start(out=wdqr_sb,
                              in_=wdqr[:, :].rearrange("(kc p) m -> p kc m", p=128))
            wdkn_sb = p2w.tile([128, KCH, 2 * ND], BF, tag="wdkn")
            nc.sync.dma_start(out=wdkn_sb,
                              in_=wdkn[:, :].rearrange("(kc p) m -> p kc m", p=128))
            wdv_sb = p2w.tile([128, KCH, 2 * VD], BF, tag="wdv")
            nc.sync.dma_start(out=wdv_sb,
                              in_=wdv[:, :].rearrange("(kc p) m -> p kc m", p=128))

            # ---- pass A: kv-dependent (gated on kv gather) ----
            for rs in range(N_CORES):
                cols = slice(rs * RPC, (rs + 1) * RPC)
                kvlat_sb = p2l.tile([128, KCH, RPC], BF, tag="kvlat")
                nc.sync.dma_start(
                    out=kvlat_sb,
                    in_=lat_kv_g[rs, 0:KCH, :, :].rearrange("c p r -> p c r"))
                krraw_sb = p2l.tile([64, RPC], BF, tag="krraw")
                nc.sync.dma_start(out=krraw_sb, in_=lat_kv_g[rs, KCH, 0:64, :])
                cs, sn = cs_sb[:, cols], sn_sb[:, cols]

                # k_nope^T cache (per head)
                for h in range(HPC):
                    pk = p2p.tile([128, RPC], F32, tag="pdec")
                    for kc in range(KCH):
                        nc.tensor.matmul(pk, wdkn_sb[:, kc, h * ND:(h + 1) * ND],
                                         kvlat_sb[:, kc, :],
                                         start=(kc == 0), stop=(kc == KCH - 1))
                    nc.scalar.activation(out=kn_cache[:, h, cols], in_=pk,
                                         func=AF.Copy)

                # k_rope: shared across heads (split to base-0 tiles first)
                krf1 = p2t.tile([32, RPC], F32, tag="krf1")
                krf2 = p2t.tile([32, RPC], F32, tag="krf2")
                nc.vector.tensor_copy(out=krf1, in_=krraw_sb[0:32, :])
                nc.vector.tensor_copy(out=krf2, in_=krraw_sb[32:64, :])
                t1 = p2t.tile([32, RPC], F32, tag="ropet")
                t2 = p2t.tile([32, RPC], F32, tag="ropet")
                nc.vector.tensor_mul(t1, krf1, cs)
                nc.vector.tensor_mul(t2, krf2, sn)
                nc.vector.tensor_sub(kr_cache[0:32, cols], t1, t2)
                nc.vector.tensor_mul(t1, krf1, sn)
                nc.vector.tensor_mul(t2, krf2, cs)
                nc.vector.tensor_add(kr_cache[32:64, cols], t1, t2)

                # v and k_nope in natural layout (lhsT = kv_lat^T chunk)
                for rc in range(RPC // 128):
                    grow = rs * RPC + rc * 128
                    pv = p2p.tile([128, 2 * VD], F32, tag="pnat")
                    for kc in range(KCH):
                        nc.tensor.matmul(pv, kvlat_sb[:, kc, rc * 128:(rc + 1) * 128],
                                         wdv_sb[:, kc, :],
                                         start=(kc == 0), stop=(kc == KCH - 1))
                    vstg = p2t.tile([128, 2 * VD], F32, tag="vstg")
                    nc.scalar.activation(out=vstg, in_=pv, func=AF.Copy)
                    nc.sync.dma_start(out=vo[grow:grow + 128, :], in_=vstg)
                    nc.vector.tensor_copy(out=v_cache[:, rs * 4 + rc, :], in_=pv)

                    pko = p2p.tile([128, 2 * ND], F32, tag="pnat")
                    for kc in range(KCH):
                        nc.tensor.matmul(pko, kvlat_sb[:, kc, rc * 128:(rc + 1) * 128],
                                         wdkn_sb[:, kc, :],
                                         start=(kc == 0), stop=(kc == KCH - 1))
                    kostg = p2t.tile([128, 2 * ND], F32, tag="vstg")
                    nc.scalar.activation(out=kostg, in_=pko, func=AF.Copy)
                    nc.sync.dma_start(out=kno[grow:grow + 128, :], in_=kostg)

            # ---- pass B: q-dependent (gated on q gather) ----
            for rs in range(N_CORES):
                cols = slice(rs * RPC, (rs + 1) * RPC)
                qlat_sb = p2l.tile([128, QCH, RPC], BF, tag="qlat")
                nc.sync.dma_start(
                    out=qlat_sb,
                    in_=lat_q_g[rs, 0:QCH, :, :].rearrange("c p r -> p c r"))
                cs, sn = cs_sb[:, cols], sn_sb[:, cols]

                for h in range(HPC):
                    pq = p2p.tile([128, RPC], F32, tag="pdec")
                    for kc in range(QCH):
                        nc.tensor.matmul(pq, wdqn_sb[:, kc, h * ND:(h + 1) * ND],
                                         qlat_sb[:, kc, :],
                                         start=(kc == 0), stop=(kc == QCH - 1))
                    nc.scalar.activation(out=qn_cache[:, h, cols], in_=pq,
                                         func=AF.Copy)

                # q_rope^T: [h0x1, h0x2, h1x1, h1x2] then rope per head
                pqr = p2p.tile([128, RPC], F32, tag="pdec")
                for kc in range(QCH):
                    nc.tensor.matmul(pqr, wdqr_sb[:, kc, :], qlat_sb[:, kc, :],
                                     start=(kc == 0), stop=(kc == QCH - 1))
                for h in range(HPC):
                    x1, x2 = pqr[h * 64:h * 64 + 32, :], pqr[h * 64 + 32:h * 64 + 64, :]
                    t1 = p2t.tile([32, RPC], F32, tag="ropet")
                    t2 = p2t.tile([32, RPC], F32, tag="ropet")
                    nc.vector.tensor_mul(t1, x1, cs)
                    nc.vector.tensor_mul(t2, x2, sn)
                    nc.vector.tensor_sub(qrc[h][0:32, cols], t1, t2)
                    nc.vector.tensor_mul(t1, x1, sn)
                    nc.vector.tensor_mul(t2, x2, cs)
                    nc.vector.tensor_add(qrc[h][32:64, cols], t1, t2)

        # ============ phase 3: attention + out projection ============
        # Transposed scores over 512-wide q-groups: scoresT[keys 128, q 512]
        # blocks; stationary operands (kn/kr/v chunks) amortize over N=512.
        # exp -> probsT bf16; denominator via ones-matmul; attn@V gives
        # outT [vd, q] = exactly the lhsT layout out-proj needs.
        NQG = T // 512
        with tc.tile_pool(name="p3w", bufs=1) as p3w, \
             tc.tile_pool(name="p3t", bufs=2) as p3t, \
             tc.tile_pool(name="p3pt", bufs=6) as p3pt, \
             tc.tile_pool(name="p3o", bufs=2) as p3o, \
             tc.tile_pool(name="ps_s", bufs=2, space="PSUM") as ps_s, \
             tc.tile_pool(name="ps_d", bufs=1, space="PSUM") as ps_d, \
             tc.tile_pool(name="ps_o", bufs=1, space="PSUM") as ps_o, \
             tc.tile_pool(name="ps_op", bufs=2, space="PSUM") as ps_op:
            wout_sb = p3w.tile([128, 2, D], BF, tag="wout")
            nc.sync.dma_start(out=wout_sb,
                              in_=wout[:, :].rearrange("(hc p) d -> p hc d", p=128))
            m4_sb = p3w.tile([128, 4, 512], F32, tag="m4")
            nc.sync.dma_start(out=m4_sb,
                              in_=masks4[:, :, :].rearrange("m p q -> p m q"))
            ones_bf = p3w.tile([128, 1], BF, tag="ones_bf")
            nc.vector.memset(ones_bf, 1.0)
            onesk1 = p3w.tile([1, 128], F32, tag="onesk1")
            nc.vector.memset(onesk1, 1.0)

            for b in range(B):
                for qg in range(NQG):
                    qcols = slice(b * T + qg * 512, b * T + (qg + 1) * 512)
                    aoqT = p3o.tile([128, HPC, 512], BF, tag="aoqT")
                    njc = 4 * (qg + 1)
                    pds = [ps_d.tile([1, 512], F32, tag=f"pd{h}",
                                      name=f"pd{h}_{b}_{qg}") for h in range(HPC)]
                    pos = [ps_o.tile([128, 512], F32, tag=f"po{h}",
                                      name=f"po{h}_{b}_{qg}") for h in range(HPC)]
                    for j in range(njc):
                        kcols = slice(b * T + j * 128, b * T + (j + 1) * 128)
                        m = j - 4 * qg
                        for h in range(HPC):
                            ps = ps_s.tile([128, 512], F32, tag="ps")
                            nc.tensor.matmul(ps, kn_cache[:, h, kcols],
                                             qn_cache[:, h, qcols],
                                             start=True, stop=False,
                                             skip_group_check=True)
                            nc.tensor.matmul(ps, kr_cache[:, kcols],
                                             qrc[h][:, qcols],
                                             start=False, stop=True,
                                             skip_group_check=True)
                            if m >= 0:
                                nc.vector.tensor_add(ps, ps, m4_sb[:, m, :])
                            pT = p3pt.tile([128, 512], BF, tag="pT")
                            nc.scalar.activation(out=pT, in_=ps, func=AF.Exp,
                                                 scale=SCALE)
                            nc.tensor.matmul(pds[h], ones_bf, pT,
                                             start=(j == 0), stop=(j == njc - 1),
                                             skip_group_check=True)
                            nc.tensor.matmul(pos[h], v_cache[:, b * NQT + j,
                                                             h * VD:(h + 1) * VD],
                                             pT,
                                             start=(j == 0), stop=(j == njc - 1),
                                             skip_group_check=True)
                    for h in range(HPC):
                        rdf = p3t.tile([1, 512], F32, tag="rdf")
                        nc.vector.reciprocal(rdf, pds[h])
                        pbc = ps_op.tile([128, 512], F32, tag="pop")
                        nc.tensor.matmul(pbc, onesk1, rdf, start=True, stop=True,
                                         skip_group_check=True)
                        bcast = p3t.tile([128, 512], F32, tag="bcast")
                        nc.scalar.activation(out=bcast, in_=pbc, func=AF.Copy)
                        nc.vector.tensor_mul(aoqT[:, h, :], pos[h], bcast)

                    # out projection for these 512 rows
                    for rc in range(4):
                        rows = slice(b * T + qg * 512 + rc * 128,
                                     b * T + qg * 512 + (rc + 1) * 128)
                        ostg = p3o.tile([128, D], F32, tag="ostg")
                        for nc4 in range(D // 512):
                            pop = ps_op.tile([128, 512], F32, tag="pop")
                            for h in range(HPC):
                                nc.tensor.matmul(
                                    pop, aoqT[:, h, rc * 128:(rc + 1) * 128],
                                    wout_sb[:, h, nc4 * 512:(nc4 + 1) * 512],
                                    start=(h == 0), stop=(h == HPC - 1))
                            if nc4 % 2 == 0:
                                nc.scalar.activation(
                                    out=ostg[:, nc4 * 512:(nc4 + 1) * 512],
                                    in_=pop, func=AF.Copy)
                            else:
                                nc.vector.tensor_copy(
                                    out=ostg[:, nc4 * 512:(nc4 + 1) * 512], in_=pop)
                        nc.sync.dma_start(out=outp[rows, :], in_=ostg)

        caches_cm.__exit__(None, None, None)

    nc.finalize()
    return nc


_PROGRAM = None


def _get_program():
    global _PROGRAM
    if _PROGRAM is None:
        _PROGRAM = build_program()
    return _PROGRAM


def run_device(in_maps, trace=False):
    _register_ntff_hook()
    from concourse.bass_utils import run_bass_kernel_spmd
    nc = _get_program()
    res = run_bass_kernel_spmd(nc, in_maps, list(range(N_CORES)), trace=trace)
    return res


def assemble(results):
    """Host-side: sum out partials, assemble k and v_t."""
    out = np.zeros((R, D), np.float32)
    for c in range(N_CORES):
        out += results[c]["outp"]
    out = out.reshape(B, T, D)

    k = np.empty((B, H, T, ND + RD), np.float32)
    v_t = np.empty((B, H, T, VD), np.float32)
    for c in range(N_CORES):
        kno = results[c]["kno"]          # [R, 2*ND]
        vo = results[c]["vo"]            # [R, 2*VD]
        for j in range(HPC):
            h = HPC * c + j
            k[:, h, :, :ND] = kno[:, j * ND:(j + 1) * ND].reshape(B, T, ND)
            v_t[:, h] = vo[:, j * VD:(j + 1) * VD].reshape(B, T, VD)

    # k rope part: gather raw slabs, undo deinterleave, rope on host (fp32)
    kr_all = np.empty((R, RD), np.float32)
    for c in range(N_CORES):
        kr_all[c * RPC:(c + 1) * RPC] = results[c]["kro"].T
    perm64 = _deint(RD)
    kr_orig = np.empty_like(kr_all)
    kr_orig[:, perm64] = kr_all          # invert permutation
    inv_freq = 1.0 / (10000.0 ** (np.arange(0, RD, 2, dtype=np.float32) / RD))
    t_of_row = np.tile(np.arange(T, dtype=np.float32), B)
    ang = t_of_row[:, None] * inv_freq[None, :]
    cos, sin = np.cos(ang), np.sin(ang)
    x1, x2 = kr_orig[:, 0::2], kr_orig[:, 1::2]
    o = np.empty_like(kr_orig)
    o[:, 0::2] = x1 * cos - x2 * sin
    o[:, 1::2] = x1 * sin + x2 * cos
    k[:, :, :, ND:] = o.reshape(B, T, RD)[:, None, :, :]
    return out, k, v_t


def kernel(**inputs):
    in_maps = host_prepare(**inputs)
    res = run_device(in_maps)
    return assemble(res.results)


# revision 20
# speedup vs baseline: 1.1137x; 1.1137x over previous
"""MLA (multi-head latent attention) forward on 8 Trainium2 NeuronCores.

Sharding: token-sharded compress (low-rank latents) + AllGather of latents +
head-sharded attention (2 heads/core) + head-sliced out-projection partials
summed on host.

All device matmuls run in bf16 with fp32 PSUM accumulation. Activations are
held transposed ([feature, token]) so every matmul contracts along partitions
with zero on-device transposes of activations (probs transposes for attn@V go
through the PE).

Self-contained: hardcodes the problem shapes from the reference
(B=2, T=2048, D=2048, H=16, ND=128, RD=64, VD=128, QR=1536, KVR=512).
"""
import math
import sys
import types

import numpy as np
import ml_dtypes

BF16 = ml_dtypes.bfloat16

# problem shapes
B, T, D = 2, 2048, 2048
H, ND, RD, VD = 16, 128, 64, 128
QR, KVR = 1536, 512
EPS = 1e-6
N_CORES = 8
HPC = H // N_CORES            # heads per core = 2
R = B * T                     # 4096 global rows
RPC = R // N_CORES            # 512 rows per core slab
NQT = T // 128                # 16 q-tiles per batch
SCALE = 1.0 / math.sqrt(ND + RD)
NEG = -30000.0

# latent chunk layout: 12 q chunks, 4 kv chunks, 1 krope chunk (64 rows used)
QCH = QR // 128               # 12
KCH = KVR // 128              # 4
LCH = QCH + KCH + 1           # 17


def _register_ntff_hook():
    try:
        import antenv
    except ImportError:
        return
    if "antenv.axon_hooks" in sys.modules:
        return
    mod = types.ModuleType("antenv.axon_hooks")
    mod._hook = None
    mod.set_axon_ntff_profile_hook = lambda h: setattr(mod, "_hook", h)
    mod.get_axon_ntff_profile_hook = lambda: mod._hook
    sys.modules["antenv.axon_hooks"] = mod
    antenv.axon_hooks = mod
    try:
        from trn_agent_boot.trn_boot import _ntff_profile_via_ctypes
        mod.set_axon_ntff_profile_hook(
            _ntff_profile_via_ctypes("/opt/axon/libaxon_pjrt.so"))
    except Exception:
        pass


def _deint(n):
    """deinterleave permutation for rope dims: [0,2,...,n-2, 1,3,...,n-1]"""
    return np.concatenate([np.arange(0, n, 2), np.arange(1, n, 2)])


def host_prepare(x, w_cq, g_q, w_ckv, g_kv, w_dq_nope, w_dq_rope,
                 w_dk_nope, w_dv, w_k_rope, w_out):
    """Build per-core input maps (numpy, bf16 for matmul operands)."""
    x = np.asarray(x, np.float32)
    xf = x.reshape(R, D)
    perm64 = _deint(RD)

    # fold rmsnorm gains into decompress weights
    wdqn_f = np.asarray(w_dq_nope, np.float32) * np.asarray(g_q, np.float32)[:, None]
    wdqr_f = np.asarray(w_dq_rope, np.float32) * np.asarray(g_q, np.float32)[:, None]
    wdkn_f = np.asarray(w_dk_nope, np.float32) * np.asarray(g_kv, np.float32)[:, None]
    wdv_f = np.asarray(w_dv, np.float32) * np.asarray(g_kv, np.float32)[:, None]

    wcq = np.asarray(w_cq, np.float32).astype(BF16)
    wckv = np.asarray(w_ckv, np.float32).astype(BF16)
    wkr = np.asarray(w_k_rope, np.float32)[:, perm64].astype(BF16)

    # rope tables, deinterleaved freq order, indexed by global row (b*T + t)
    inv_freq = 1.0 / (10000.0 ** (np.arange(0, RD, 2, dtype=np.float32) / RD))  # [32]
    t_of_row = np.tile(np.arange(T, dtype=np.float32), B)                       # [R]
    ang = inv_freq[:, None] * t_of_row[None, :]                                 # [32, R]
    cos32 = np.cos(ang).astype(np.float32)
    sin32 = np.sin(ang).astype(np.float32)

    # transposed causal masks for 512-wide q-groups: for relative key chunk m
    # (0..3), maskT[m][kr, qc] = 0 if (m*128 + kr) <= qc else NEG
    kr = np.arange(128)[:, None]
    qc = np.arange(512)[None, :]
    masks4 = np.stack([np.where(m * 128 + kr <= qc, 0.0, NEG)
                       for m in range(4)]).astype(np.float32)

    in_maps = []
    for c in range(N_CORES):
        h0, h1 = 2 * c, 2 * c + 1
        # w_dq_rope columns for head h: [h*RD, (h+1)*RD); per-head [even32, odd32]
        qr_cols = np.concatenate([h0 * RD + perm64, h1 * RD + perm64])
        nope_cols = np.concatenate(
            [np.arange(h0 * ND, (h0 + 1) * ND), np.arange(h1 * ND, (h1 + 1) * ND)])
        v_cols = np.concatenate(
            [np.arange(h0 * VD, (h0 + 1) * VD), np.arange(h1 * VD, (h1 + 1) * VD)])
        in_maps.append({
            "xt": np.ascontiguousarray(xf[c * RPC:(c + 1) * RPC].T).astype(BF16),
            "wcq": wcq,
            "wckv": wckv,
            "wkr": wkr,
            "wdqn": wdqn_f[:, nope_cols].astype(BF16),
            "wdqr": wdqr_f[:, qr_cols].astype(BF16),
            "wdkn": wdkn_f[:, nope_cols].astype(BF16),
            "wdv": wdv_f[:, v_cols].astype(BF16),
            "wout": np.ascontiguousarray(
                np.asarray(w_out, np.float32)[v_cols, :]).astype(BF16),
            "cos32": cos32,
            "sin32": sin32,
            "masks4": masks4,
        })
    return in_maps


def build_program():
    import concourse.bass as bass
    import concourse.tile as tile
    from concourse import bacc, mybir

    dt = mybir.dt
    F32, BF = dt.float32, dt.bfloat16
    AF = mybir.ActivationFunctionType

    nc = bacc.Bacc(None, target_bir_lowering=False)

    # ---- I/O ----
    xt = nc.declare_dram_parameter("xt", [D, RPC], BF, isOutput=False)
    wcq = nc.declare_dram_parameter("wcq", [D, QR], BF, isOutput=False)
    wckv = nc.declare_dram_parameter("wckv", [D, KVR], BF, isOutput=False)
    wkr = nc.declare_dram_parameter("wkr", [D, RD], BF, isOutput=False)
    wdqn = nc.declare_dram_parameter("wdqn", [QR, 2 * ND], BF, isOutput=False)
    wdqr = nc.declare_dram_parameter("wdqr", [QR, 2 * RD], BF, isOutput=False)
    wdkn = nc.declare_dram_parameter("wdkn", [KVR, 2 * ND], BF, isOutput=False)
    wdv = nc.declare_dram_parameter("wdv", [KVR, 2 * VD], BF, isOutput=False)
    wout = nc.declare_dram_parameter("wout", [2 * VD, D], BF, isOutput=False)
    cos32 = nc.declare_dram_parameter("cos32", [RD // 2, R], F32, isOutput=False)
    sin32 = nc.declare_dram_parameter("sin32", [RD // 2, R], F32, isOutput=False)
    masks4 = nc.declare_dram_parameter("masks4", [4, 128, 512], F32, isOutput=False)

    kno = nc.declare_dram_parameter("kno", [R, 2 * ND], F32, isOutput=True)
    vo = nc.declare_dram_parameter("vo", [R, 2 * VD], F32, isOutput=True)
    kro = nc.declare_dram_parameter("kro", [RD, RPC], F32, isOutput=True)
    outp = nc.declare_dram_parameter("outp", [R, D], F32, isOutput=True)

    lat_kv_loc = nc.dram_tensor("lat_kv_loc", [KCH + 1, 128, RPC], BF)
    lat_q_loc = nc.dram_tensor("lat_q_loc", [QCH, 128, RPC], BF)
    lat_kv_g = nc.dram_tensor("lat_kv_g", [N_CORES, KCH + 1, 128, RPC], BF,
                              addr_space="Shared")
    lat_q_g = nc.dram_tensor("lat_q_g", [N_CORES, QCH, 128, RPC], BF,
                             addr_space="Shared")

    with tile.TileContext(nc) as tc:
        # ================= phase 1: compress own slab =================
        with tc.tile_pool(name="p1w", bufs=1) as p1w, \
             tc.tile_pool(name="p1s", bufs=1) as p1s, \
             tc.tile_pool(name="p1t", bufs=3) as p1t, \
             tc.tile_pool(name="p1p", bufs=2, space="PSUM") as p1p, \
             tc.tile_pool(name="p1pk", bufs=1, space="PSUM") as p1pk, \
             tc.tile_pool(name="p1q", bufs=1, space="PSUM") as p1q:
            xt_sb = p1w.tile([128, D // 128, RPC], BF, tag="xt")
            nc.sync.dma_start(out=xt_sb,
                              in_=xt[:, :].rearrange("(kc p) r -> p kc r", p=128))
            wckv_sb = p1w.tile([128, D // 128, KVR], BF, tag="wckv")
            nc.sync.dma_start(out=wckv_sb,
                              in_=wckv[:, :].rearrange("(kc p) m -> p kc m", p=128))
            wkr_sb = p1w.tile([128, D // 128, RD], BF, tag="wkr")
            nc.sync.dma_start(out=wkr_sb,
                              in_=wkr[:, :].rearrange("(kc p) m -> p kc m", p=128))
            wcq_sb = p1w.tile([128, D // 128, QR], BF, tag="wcq")
            nc.sync.dma_start(out=wcq_sb,
                              in_=wcq[:, :].rearrange("(kc p) m -> p kc m", p=128))
            ones128 = p1w.tile([128, 1], BF, tag="ones128")
            nc.vector.memset(ones128, 1.0)
            ones_k1 = p1w.tile([1, 128], F32, tag="ones_k1")
            nc.vector.memset(ones_k1, 1.0)
            eps_sb = p1w.tile([1, 1], F32, tag="eps_sb")
            nc.vector.memset(eps_sb, EPS)

            qstg = p1s.tile([128, QCH, RPC], F32, tag="qstg")
            kstg = p1s.tile([128, KCH, RPC], F32, tag="kstg")
            latq_sb = p1s.tile([128, QCH, RPC], BF, tag="latq_sb")
            latkv_sb = p1s.tile([128, KCH + 1, RPC], BF, tag="latkv_sb")

            psq_q = p1q.tile([1, RPC], F32, tag="psq_q")
            psq_k = p1q.tile([1, RPC], F32, tag="psq_k")

            def compress(n_ch, w_sb, stg, psq):
                for j in range(n_ch):
                    pc = p1p.tile([128, RPC], F32, tag="pc")
                    for kc in range(D // 128):
                        nc.tensor.matmul(pc, w_sb[:, kc, j * 128:(j + 1) * 128],
                                         xt_sb[:, kc, :],
                                         start=(kc == 0), stop=(kc == D // 128 - 1))
                    nc.scalar.activation(out=stg[:, j, :], in_=pc, func=AF.Copy)
                    sq = p1t.tile([128, RPC], BF, tag="sq")
                    nc.scalar.activation(out=sq, in_=pc, func=AF.Square)
                    nc.tensor.matmul(psq, ones128, sq,
                                     start=(j == 0), stop=(j == n_ch - 1),
                                     skip_group_check=True)

            def normalize(stg, psq, n_ch, dim, dst_sb, dst_off):
                tsd = p1t.tile([1, RPC], F32, tag="tsd")
                nc.scalar.activation(out=tsd, in_=psq, func=AF.Sqrt,
                                     scale=1.0 / dim, bias=eps_sb)
                rstd = p1t.tile([1, RPC], F32, tag="rstd")
                nc.vector.reciprocal(rstd, tsd)
                pbc = p1pk.tile([128, RPC], F32, tag="pbc")
                nc.tensor.matmul(pbc, ones_k1, rstd, start=True, stop=True)
                bcast = p1t.tile([128, RPC], F32, tag="bcast")
                nc.scalar.activation(out=bcast, in_=pbc, func=AF.Copy)
                for j in range(n_ch):
                    nc.vector.tensor_mul(dst_sb[:, dst_off + j, :],
                                         stg[:, j, :], bcast)

            # ---- kv latents + k_rope first, gather early ----
            compress(KCH, wckv_sb, kstg, psq_k)
            pkr = p1pk.tile([64, RPC], F32, tag="pkr")
            for kc in range(D // 128):
                nc.tensor.matmul(pkr, wkr_sb[:, kc, :], xt_sb[:, kc, :],
                                 start=(kc == 0), stop=(kc == D // 128 - 1))
            krstg = p1t.tile([64, RPC], F32, tag="krstg")
            nc.scalar.activation(out=krstg, in_=pkr, func=AF.Copy)
            nc.sync.dma_start(out=kro[:, :], in_=krstg)
            nc.vector.tensor_copy(out=latkv_sb[0:64, KCH, :], in_=krstg)
            nc.vector.memset(latkv_sb[64:128, KCH, :], 0.0)
            normalize(kstg, psq_k, KCH, KVR, latkv_sb, 0)
            nc.sync.dma_start(
                out=lat_kv_loc[:, :, :].rearrange("c p r -> p c r"), in_=latkv_sb)
            with tc.tile_critical():
                with nc.semaphore() as csem1:
                    nc.gpsimd.collective_compute(
                        "AllGather", mybir.AluOpType.bypass,
                        replica_groups=[list(range(N_CORES))],
                        ins=[lat_kv_loc[:]], outs=[lat_kv_g[:]],
                    ).then_inc(csem1, 1)
                    nc.gpsimd.wait_ge(csem1, 1)

            # ---- q latents, gather second ----
            compress(QCH, wcq_sb, qstg, psq_q)
            normalize(qstg, psq_q, QCH, QR, latq_sb, 0)
            nc.sync.dma_start(
                out=lat_q_loc[:, :, :].rearrange("c p r -> p c r"), in_=latq_sb)
            with tc.tile_critical():
                with nc.semaphore() as csem2:
                    nc.gpsimd.collective_compute(
                        "AllGather", mybir.AluOpType.bypass,
                        replica_groups=[list(range(N_CORES))],
                        ins=[lat_q_loc[:]], outs=[lat_q_g[:]],
                    ).then_inc(csem2, 1)
                    nc.gpsimd.wait_ge(csem2, 1)

        # ============ phase 2: decompress q/k/v for own heads ============
        caches_cm = tc.tile_pool(name="caches", bufs=1)
        caches = caches_cm.__enter__()
        qn_cache = caches.tile([128, HPC, R], BF, tag="qn_cache")
        qrc0 = caches.tile([64, R], BF, tag="qrc0")
        qrc1 = caches.tile([64, R], BF, tag="qrc1")
        qrc = [qrc0, qrc1]
        kn_cache = caches.tile([128, HPC, R], BF, tag="kn_cache")
        kr_cache = caches.tile([64, R], BF, tag="kr_cache")
        v_cache = caches.tile([128, R // 128, 2 * VD], BF, tag="v_cache")
        cs_sb = caches.tile([32, R], F32, tag="cs_sb")
        sn_sb = caches.tile([32, R], F32, tag="sn_sb")
        nc.sync.dma_start(out=cs_sb, in_=cos32[:, :])
        nc.sync.dma_start(out=sn_sb, in_=sin32[:, :])

        with tc.tile_pool(name="p2w", bufs=1) as p2w, \
             tc.tile_pool(name="p2l", bufs=2) as p2l, \
             tc.tile_pool(name="p2t", bufs=4) as p2t, \
             tc.tile_pool(name="p2p", bufs=4, space="PSUM") as p2p:
            wdqn_sb = p2w.tile([128, QCH, 2 * ND], BF, tag="wdqn")
            nc.sync.dma_start(out=wdqn_sb,
                              in_=wdqn[:, :].rearrange("(kc p) m -> p kc m", p=128))
            wdqr_sb = p2w.tile([128, QCH, 2 * RD], BF, tag="wdqr")
            nc.sync.dma_start(out=wdqr_sb,
                              in_=wdqr[:, :].rearrange("(kc p) m -> p kc m", p=128))
            wdkn_sb = p2w.tile([128, KCH, 2 * ND], BF, tag="wdkn")
            nc.sync.dma_start(out=wdkn_sb,
                              in_=wdkn[:, :].rearrange("(kc p) m -> p kc m", p=128))
            wdv_sb = p2w.tile([128, KCH, 2 * VD], BF, tag="wdv")
            nc.sync.dma_start(out=wdv_sb,
                              in_=wdv[:, :].rearrange("(kc p) m -> p kc m", p=128))

            # ---- pass A: kv-dependent (gated on kv gather) ----
            for rs in range(N_CORES):
                cols = slice(rs * RPC, (rs + 1) * RPC)
                kvlat_sb = p2l.tile([128, KCH, RPC], BF, tag="kvlat")
                nc.sync.dma_start(
                    out=kvlat_sb,
                    in_=lat_kv_g[rs, 0:KCH, :, :].rearrange("c p r -> p c r"))
                krraw_sb = p2l.tile([64, RPC], BF, tag="krraw")
                nc.sync.dma_start(out=krraw_sb, in_=lat_kv_g[rs, KCH, 0:64, :])
                cs, sn = cs_sb[:, cols], sn_sb[:, cols]

                # k_nope^T cache (per head)
                for h in range(HPC):
                    pk = p2p.tile([128, RPC], F32, tag="pdec")
                    for kc in range(KCH):
                        nc.tensor.matmul(pk, wdkn_sb[:, kc, h * ND:(h + 1) * ND],
                                         kvlat_sb[:, kc, :],
                                         start=(kc == 0), stop=(kc == KCH - 1))
                    nc.scalar.activation(out=kn_cache[:, h, cols], in_=pk,
                                         func=AF.Copy)

                # k_rope: shared across heads (split to base-0 tiles first)
                krf1 = p2t.tile([32, RPC], F32, tag="krf1")
                krf2 = p2t.tile([32, RPC], F32, tag="krf2")
                nc.vector.tensor_copy(out=krf1, in_=krraw_sb[0:32, :])
                nc.vector.tensor_copy(out=krf2, in_=krraw_sb[32:64, :])
                t1 = p2t.tile([32, RPC], F32, tag="ropet")
                t2 = p2t.tile([32, RPC], F32, tag="ropet")
                nc.vector.tensor_mul(t1, krf1, cs)
                nc.vector.tensor_mul(t2, krf2, sn)
                nc.vector.tensor_sub(kr_cache[0:32, cols], t1, t2)
                nc.vector.tensor_mul(t1, krf1, sn)
                nc.vector.tensor_mul(t2, krf2, cs)
                nc.vector.tensor_add(kr_cache[32:64, cols], t1, t2)

                # v and k_nope in natural layout (lhsT = kv_lat^T chunk)
                for rc in range(RPC // 128):
                    grow = rs * RPC + rc * 128
                    pv = p2p.tile([128, 2 * VD], F32, tag="pnat")
                    for kc in range(KCH):
                        nc.tensor.matmul(pv, kvlat_sb[:, kc, rc * 128:(rc + 1) * 128],
                                         wdv_sb[:, kc, :],
                                         start=(kc == 0), stop=(kc == KCH - 1))
                    vstg = p2t.tile([128, 2 * VD], F32, tag="vstg")
                    nc.scalar.activation(out=vstg, in_=pv, func=AF.Copy)
                    nc.sync.dma_start(out=vo[grow:grow + 128, :], in_=vstg)
                    nc.vector.tensor_copy(out=v_cache[:, rs * 4 + rc, :], in_=pv)

                    pko = p2p.tile([128, 2 * ND], F32, tag="pnat")
                    for kc in range(KCH):
                        nc.tensor.matmul(pko, kvlat_sb[:, kc, rc * 128:(rc + 1) * 128],
                                         wdkn_sb[:, kc, :],
                                         start=(kc == 0), stop=(kc == KCH - 1))
                    kostg = p2t.tile([128, 2 * ND], F32, tag="vstg")
                    nc.scalar.activation(out=kostg, in_=pko, func=AF.Copy)
                    nc.sync.dma_start(out=kno[grow:grow + 128, :], in_=kostg)

            # ---- pass B: q-dependent (gated on q gather) ----
            for rs in range(N_CORES):
                cols = slice(rs * RPC, (rs + 1) * RPC)
                qlat_sb = p2l.tile([128, QCH, RPC], BF, tag="qlat")
                nc.sync.dma_start(
                    out=qlat_sb,
                    in_=lat_q_g[rs, 0:QCH, :, :].rearrange("c p r -> p c r"))
                cs, sn = cs_sb[:, cols], sn_sb[:, cols]

                for h in range(HPC):
                    pq = p2p.tile([128, RPC], F32, tag="pdec")
                    for kc in range(QCH):
                        nc.tensor.matmul(pq, wdqn_sb[:, kc, h * ND:(h + 1) * ND],
                                         qlat_sb[:, kc, :],
                                         start=(kc == 0), stop=(kc == QCH - 1))
                    nc.scalar.activation(out=qn_cache[:, h, cols], in_=pq,
                                         func=AF.Copy)

                # q_rope^T: [h0x1, h0x2, h1x1, h1x2] then rope per head
                pqr = p2p.tile([128, RPC], F32, tag="pdec")
                for kc in range(QCH):
                    nc.tensor.matmul(pqr, wdqr_sb[:, kc, :], qlat_sb[:, kc, :],
                                     start=(kc == 0), stop=(kc == QCH - 1))
                for h in range(HPC):
                    x1, x2 = pqr[h * 64:h * 64 + 32, :], pqr[h * 64 + 32:h * 64 + 64, :]
                    t1 = p2t.tile([32, RPC], F32, tag="ropet")
                    t2 = p2t.tile([32, RPC], F32, tag="ropet")
                    nc.vector.tensor_mul(t1, x1, cs)
                    nc.vector.tensor_mul(t2, x2, sn)
                    nc.vector.tensor_sub(qrc[h][0:32, cols], t1, t2)
                    nc.vector.tensor_mul(t1, x1, sn)
                    nc.vector.tensor_mul(t2, x2, cs)
                    nc.vector.tensor_add(qrc[h][32:64, cols], t1, t2)

        # ============ phase 3: attention + out projection ============
        # Transposed scores over 512-wide q-groups: scoresT[keys 128, q 512]
        # blocks; stationary operands (kn/kr/v chunks) amortize over N=512.
        # exp -> probsT bf16; denominator via ones-matmul; attn@V gives
        # outT [vd, q] = exactly the lhsT layout out-proj needs.
        NQG = T // 512
        with tc.tile_pool(name="p3w", bufs=1) as p3w, \
             tc.tile_pool(name="p3t", bufs=2) as p3t, \
             tc.tile_pool(name="p3pt", bufs=2) as p3pt, \
             tc.tile_pool(name="p3o", bufs=2) as p3o, \
             tc.tile_pool(name="ps_s", bufs=3, space="PSUM") as ps_s, \
             tc.tile_pool(name="ps_d", bufs=1, space="PSUM") as ps_d, \
             tc.tile_pool(name="ps_o", bufs=2, space="PSUM") as ps_o, \
             tc.tile_pool(name="ps_op", bufs=2, space="PSUM") as ps_op:
            wout_sb = p3w.tile([128, 2, D], BF, tag="wout")
            nc.sync.dma_start(out=wout_sb,
                              in_=wout[:, :].rearrange("(hc p) d -> p hc d", p=128))
            m4_sb = p3w.tile([128, 4, 512], F32, tag="m4")
            nc.sync.dma_start(out=m4_sb,
                              in_=masks4[:, :, :].rearrange("m p q -> p m q"))
            ones_bf = p3w.tile([128, 1], BF, tag="ones_bf")
            nc.vector.memset(ones_bf, 1.0)
            onesk1 = p3w.tile([1, 128], F32, tag="onesk1")
            nc.vector.memset(onesk1, 1.0)

            for b in range(B):
                for qg in range(NQG):
                    qcols = slice(b * T + qg * 512, b * T + (qg + 1) * 512)
                    aoqT = p3o.tile([128, HPC, 512], BF, tag="aoqT")
                    njc = 4 * (qg + 1)
                    for h in range(HPC):
                        pT_all = p3pt.tile([128, NQT, 512], BF, tag="pT",
                                           name=f"pT_{b}_{qg}_{h}")
                        for j in range(njc):
                            kcols = slice(b * T + j * 128, b * T + (j + 1) * 128)
                            ps = ps_s.tile([128, 512], F32, tag="ps")
                            nc.tensor.matmul(ps, kn_cache[:, h, kcols],
                                             qn_cache[:, h, qcols],
                                             start=True, stop=False,
                                             skip_group_check=True)
                            nc.tensor.matmul(ps, kr_cache[:, kcols],
                                             qrc[h][:, qcols],
                                             start=False, stop=True,
                                             skip_group_check=True)
                            m = j - 4 * qg
                            if m >= 0:
                                nc.vector.tensor_add(ps, ps, m4_sb[:, m, :])
                            nc.scalar.activation(out=pT_all[:, j, :], in_=ps,
                                                 func=AF.Exp, scale=SCALE)
                        pd = ps_d.tile([1, 512], F32, tag="pd")
                        for j in range(njc):
                            nc.tensor.matmul(pd, ones_bf, pT_all[:, j, :],
                                             start=(j == 0), stop=(j == njc - 1),
                                             skip_group_check=True)
                        po = ps_o.tile([128, 512], F32, tag="po")
                        for j in range(njc):
                            nc.tensor.matmul(po, v_cache[:, b * NQT + j,
                                                         h * VD:(h + 1) * VD],
                                             pT_all[:, j, :],
                                             start=(j == 0), stop=(j == njc - 1),
                                             skip_group_check=True)
                        rdf = p3t.tile([1, 512], F32, tag="rdf")
                        nc.vector.reciprocal(rdf, pd)
                        pbc = ps_op.tile([128, 512], F32, tag="pop")
                        nc.tensor.matmul(pbc, onesk1, rdf, start=True, stop=True,
                                         skip_group_check=True)
                        bcast = p3t.tile([128, 512], F32, tag="bcast")
                        nc.scalar.activation(out=bcast, in_=pbc, func=AF.Copy)
                        nc.vector.tensor_mul(aoqT[:, h, :], po, bcast)

                    # out projection for these 512 rows
                    for rc in range(4):
                        rows = slice(b * T + qg * 512 + rc * 128,
                                     b * T + qg * 512 + (rc + 1) * 128)
                        ostg = p3o.tile([128, D], F32, tag="ostg")
                        for nc4 in range(D // 512):
                            pop = ps_op.tile([128, 512], F32, tag="pop")
                            for h in range(HPC):
                                nc.tensor.matmul(
                                    pop, aoqT[:, h, rc * 128:(rc + 1) * 128],
                                    wout_sb[:, h, nc4 * 512:(nc4 + 1) * 512],
                                    start=(h == 0), stop=(h == HPC - 1))
                            if nc4 % 2 == 0:
                                nc.scalar.activation(
                                    out=ostg[:, nc4 * 512:(nc4 + 1) * 512],
                                    in_=pop, func=AF.Copy)
                            else:
                                nc.vector.tensor_copy(
                                    out=ostg[:, nc4 * 512:(nc4 + 1) * 512], in_=pop)
                        nc.sync.dma_start(out=outp[rows, :], in_=ostg)

        caches_cm.__exit__(None, None, None)

    nc.finalize()
    return nc


_PROGRAM = None


def _get_program():
    global _PROGRAM
    if _PROGRAM is None:
        _PROGRAM = build_program()
    return _PROGRAM


def run_device(in_maps, trace=False):
    _register_ntff_hook()
    from concourse.bass_utils import run_bass_kernel_spmd
    nc = _get_program()
    res = run_bass_kernel_spmd(nc, in_maps, list(range(N_CORES)), trace=trace)
    return res


def assemble(results):
    """Host-side: sum out partials, assemble k and v_t."""
    out = np.zeros((R, D), np.float32)
    for c in range(N_CORES):
        out += results[c]["outp"]
    out = out.reshape(B, T, D)

    k = np.empty((B, H, T, ND + RD), np.float32)
    v_t = np.empty((B, H, T, VD), np.float32)
    for c in range(N_CORES):
        kno = results[c]["kno"]          # [R, 2*ND]
        vo = results[c]["vo"]            # [R, 2*VD]
        for j in range(HPC):
            h = HPC * c + j
            k[:, h, :, :ND] = kno[:, j * ND:(j + 1) * ND].reshape(B, T, ND)
            v_t[:, h] = vo[:, j * VD:(j + 1) * VD].reshape(B, T, VD)

    # k rope part: gather raw slabs, undo deinterleave, rope on host (fp32)
    kr_all = np.empty((R, RD), np.float32)
    for c in range(N_CORES):
        kr_all[c * RPC:(c + 1) * RPC] = results[c]["kro"].T
    perm64 = _deint(RD)
    kr_orig = np.empty_like(kr_all)
    kr_orig[:, perm64] = kr_all          # invert permutation
    inv_freq = 1.0 / (10000.0 ** (np.arange(0, RD, 2, dtype=np.float32) / RD))
    t_of_row = np.tile(np.arange(T, dtype=np.float32), B)
    ang = t_of_row[:, None] * inv_freq[None, :]
    cos, sin = np.cos(ang), np.sin(ang)
    x1, x2 = kr_orig[:, 0::2], kr_orig[:, 1::2]
    o = np.empty_like(kr_orig)
    o[:, 0::2] = x1 * cos - x2 * sin
    o[:, 1::2] = x1 * sin + x2 * cos
    k[:, :, :, ND:] = o.reshape(B, T, RD)[:, None, :, :]
    return out, k, v_t


def kernel(**inputs):
    in_maps = host_prepare(**inputs)
    res = run_device(in_maps)
    return assemble(res.results)


# revision 24
# speedup vs baseline: 1.1764x; 1.0563x over previous
"""MLA (multi-head latent attention) forward on 8 Trainium2 NeuronCores.

Sharding: token-sharded compress (low-rank latents) + AllGather of latents +
head-sharded attention (2 heads/core) + head-sliced out-projection partials
summed on host.

All device matmuls run in bf16 with fp32 PSUM accumulation. Activations are
held transposed ([feature, token]) so every matmul contracts along partitions
with zero on-device transposes of activations (probs transposes for attn@V go
through the PE).

Self-contained: hardcodes the problem shapes from the reference
(B=2, T=2048, D=2048, H=16, ND=128, RD=64, VD=128, QR=1536, KVR=512).
"""
import math
import sys
import types

import numpy as np
import ml_dtypes

BF16 = ml_dtypes.bfloat16

# problem shapes
B, T, D = 2, 2048, 2048
H, ND, RD, VD = 16, 128, 64, 128
QR, KVR = 1536, 512
EPS = 1e-6
N_CORES = 8
HPC = H // N_CORES            # heads per core = 2
R = B * T                     # 4096 global rows
RPC = R // N_CORES            # 512 rows per core slab
NQT = T // 128                # 16 q-tiles per batch
SCALE = 1.0 / math.sqrt(ND + RD)
NEG = -30000.0

# latent chunk layout: 12 q chunks, 4 kv chunks, 1 krope chunk (64 rows used)
QCH = QR // 128               # 12
KCH = KVR // 128              # 4
LCH = QCH + KCH + 1           # 17


def _register_ntff_hook():
    try:
        import antenv
    except ImportError:
        return
    if "antenv.axon_hooks" in sys.modules:
        return
    mod = types.ModuleType("antenv.axon_hooks")
    mod._hook = None
    mod.set_axon_ntff_profile_hook = lambda h: setattr(mod, "_hook", h)
    mod.get_axon_ntff_profile_hook = lambda: mod._hook
    sys.modules["antenv.axon_hooks"] = mod
    antenv.axon_hooks = mod
    try:
        from trn_agent_boot.trn_boot import _ntff_profile_via_ctypes
        mod.set_axon_ntff_profile_hook(
            _ntff_profile_via_ctypes("/opt/axon/libaxon_pjrt.so"))
    except Exception:
        pass


def _deint(n):
    """deinterleave permutation for rope dims: [0,2,...,n-2, 1,3,...,n-1]"""
    return np.concatenate([np.arange(0, n, 2), np.arange(1, n, 2)])


def host_prepare(x, w_cq, g_q, w_ckv, g_kv, w_dq_nope, w_dq_rope,
                 w_dk_nope, w_dv, w_k_rope, w_out):
    """Build per-core input maps (numpy, bf16 for matmul operands)."""
    x = np.asarray(x, np.float32)
    xf = x.reshape(R, D)
    perm64 = _deint(RD)

    # fold rmsnorm gains into decompress weights
    wdqn_f = np.asarray(w_dq_nope, np.float32) * np.asarray(g_q, np.float32)[:, None]
    wdqr_f = np.asarray(w_dq_rope, np.float32) * np.asarray(g_q, np.float32)[:, None]
    wdkn_f = np.asarray(w_dk_nope, np.float32) * np.asarray(g_kv, np.float32)[:, None]
    wdv_f = np.asarray(w_dv, np.float32) * np.asarray(g_kv, np.float32)[:, None]

    wcq = np.asarray(w_cq, np.float32).astype(BF16)
    wckv = np.asarray(w_ckv, np.float32).astype(BF16)
    wkr = np.asarray(w_k_rope, np.float32)[:, perm64].astype(BF16)

    # rope tables, deinterleaved freq order, indexed by global row (b*T + t)
    inv_freq = 1.0 / (10000.0 ** (np.arange(0, RD, 2, dtype=np.float32) / RD))  # [32]
    t_of_row = np.tile(np.arange(T, dtype=np.float32), B)                       # [R]
    ang = inv_freq[:, None] * t_of_row[None, :]                                 # [32, R]
    cos32 = np.cos(ang).astype(np.float32)
    sin32 = np.sin(ang).astype(np.float32)

    # transposed causal masks for 512-wide q-groups: for relative key chunk m
    # (0..3), maskT[m][kr, qc] = 0 if (m*128 + kr) <= qc else NEG
    kr = np.arange(128)[:, None]
    qc = np.arange(512)[None, :]
    masks4 = np.stack([np.where(m * 128 + kr <= qc, 0.0, NEG)
                       for m in range(4)]).astype(np.float32)

    in_maps = []
    for c in range(N_CORES):
        h0, h1 = 2 * c, 2 * c + 1
        # w_dq_rope columns for head h: [h*RD, (h+1)*RD); per-head [even32, odd32]
        qr_cols = np.concatenate([h0 * RD + perm64, h1 * RD + perm64])
        nope_cols = np.concatenate(
            [np.arange(h0 * ND, (h0 + 1) * ND), np.arange(h1 * ND, (h1 + 1) * ND)])
        v_cols = np.concatenate(
            [np.arange(h0 * VD, (h0 + 1) * VD), np.arange(h1 * VD, (h1 + 1) * VD)])
        in_maps.append({
            "xt": np.ascontiguousarray(xf[c * RPC:(c + 1) * RPC].T).astype(BF16),
            "wcq": wcq,
            "wckv": wckv,
            "wkr": wkr,
            "wdqn": wdqn_f[:, nope_cols].astype(BF16),
            "wdqr": wdqr_f[:, qr_cols].astype(BF16),
            "wdkn": wdkn_f[:, nope_cols].astype(BF16),
            "wdv": wdv_f[:, v_cols].astype(BF16),
            "wout": np.ascontiguousarray(
                np.asarray(w_out, np.float32)[v_cols, :]).astype(BF16),
            "cos32": cos32,
            "sin32": sin32,
            "masks4": masks4,
        })
    return in_maps


def build_program():
    import concourse.bass as bass
    import concourse.tile as tile
    from concourse import bacc, mybir

    dt = mybir.dt
    F32, BF = dt.float32, dt.bfloat16
    AF = mybir.ActivationFunctionType

    nc = bacc.Bacc(None, target_bir_lowering=False)

    # ---- I/O ----
    xt = nc.declare_dram_parameter("xt", [D, RPC], BF, isOutput=False)
    wcq = nc.declare_dram_parameter("wcq", [D, QR], BF, isOutput=False)
    wckv = nc.declare_dram_parameter("wckv", [D, KVR], BF, isOutput=False)
    wkr = nc.declare_dram_parameter("wkr", [D, RD], BF, isOutput=False)
    wdqn = nc.declare_dram_parameter("wdqn", [QR, 2 * ND], BF, isOutput=False)
    wdqr = nc.declare_dram_parameter("wdqr", [QR, 2 * RD], BF, isOutput=False)
    wdkn = nc.declare_dram_parameter("wdkn", [KVR, 2 * ND], BF, isOutput=False)
    wdv = nc.declare_dram_parameter("wdv", [KVR, 2 * VD], BF, isOutput=False)
    wout = nc.declare_dram_parameter("wout", [2 * VD, D], BF, isOutput=False)
    cos32 = nc.declare_dram_parameter("cos32", [RD // 2, R], F32, isOutput=False)
    sin32 = nc.declare_dram_parameter("sin32", [RD // 2, R], F32, isOutput=False)
    masks4 = nc.declare_dram_parameter("masks4", [4, 128, 512], F32, isOutput=False)

    kno = nc.declare_dram_parameter("kno", [R, 2 * ND], F32, isOutput=True)
    vo = nc.declare_dram_parameter("vo", [R, 2 * VD], F32, isOutput=True)
    kro = nc.declare_dram_parameter("kro", [RD, RPC], F32, isOutput=True)
    outp = nc.declare_dram_parameter("outp", [R, D], F32, isOutput=True)

    lat_kv_loc = nc.dram_tensor("lat_kv_loc", [KCH + 1, 128, RPC], BF)
    lat_q_loc = nc.dram_tensor("lat_q_loc", [QCH, 128, RPC], BF)
    lat_kv_g = nc.dram_tensor("lat_kv_g", [N_CORES, KCH + 1, 128, RPC], BF,
                              addr_space="Shared")
    lat_q_g = nc.dram_tensor("lat_q_g", [N_CORES, QCH, 128, RPC], BF,
                             addr_space="Shared")

    with tile.TileContext(nc) as tc:
        # ================= phase 1: compress own slab =================
        with tc.tile_pool(name="p1w", bufs=1) as p1w, \
             tc.tile_pool(name="p1s", bufs=1) as p1s, \
             tc.tile_pool(name="p1t", bufs=3) as p1t, \
             tc.tile_pool(name="p1p", bufs=2, space="PSUM") as p1p, \
             tc.tile_pool(name="p1pk", bufs=1, space="PSUM") as p1pk, \
             tc.tile_pool(name="p1q", bufs=1, space="PSUM") as p1q:
            xt_sb = p1w.tile([128, D // 128, RPC], BF, tag="xt")
            nc.sync.dma_start(out=xt_sb,
                              in_=xt[:, :].rearrange("(kc p) r -> p kc r", p=128))
            wckv_sb = p1w.tile([128, D // 128, KVR], BF, tag="wckv")
            nc.sync.dma_start(out=wckv_sb,
                              in_=wckv[:, :].rearrange("(kc p) m -> p kc m", p=128))
            wkr_sb = p1w.tile([128, D // 128, RD], BF, tag="wkr")
            nc.sync.dma_start(out=wkr_sb,
                              in_=wkr[:, :].rearrange("(kc p) m -> p kc m", p=128))
            wcq_sb = p1w.tile([128, D // 128, QR], BF, tag="wcq")
            nc.sync.dma_start(out=wcq_sb,
                              in_=wcq[:, :].rearrange("(kc p) m -> p kc m", p=128))
            ones128 = p1w.tile([128, 1], BF, tag="ones128")
            nc.vector.memset(ones128, 1.0)
            eps_sb = p1w.tile([1, 1], F32, tag="eps_sb")
            nc.vector.memset(eps_sb, EPS)

            qstg = p1s.tile([128, QCH, RPC], F32, tag="qstg")
            kstg = p1s.tile([128, KCH, RPC], F32, tag="kstg")
            latq_sb = p1s.tile([128, QCH, RPC], BF, tag="latq_sb")
            latkv_sb = p1s.tile([128, KCH + 1, RPC], BF, tag="latkv_sb")

            psq_q = p1q.tile([1, RPC], F32, tag="psq_q")
            psq_k = p1q.tile([1, RPC], F32, tag="psq_k")

            def compress(n_ch, w_sb, stg, psq):
                for j in range(n_ch):
                    pc = p1p.tile([128, RPC], F32, tag="pc")
                    for kc in range(D // 128):
                        nc.tensor.matmul(pc, w_sb[:, kc, j * 128:(j + 1) * 128],
                                         xt_sb[:, kc, :],
                                         start=(kc == 0), stop=(kc == D // 128 - 1))
                    nc.scalar.activation(out=stg[:, j, :], in_=pc, func=AF.Copy)
                    sq = p1t.tile([128, RPC], BF, tag="sq")
                    nc.scalar.activation(out=sq, in_=pc, func=AF.Square)
                    nc.tensor.matmul(psq, ones128, sq,
                                     start=(j == 0), stop=(j == n_ch - 1),
                                     skip_group_check=True)

            def normalize(stg, psq, n_ch, dim, dst_sb, dst_off):
                tsd = p1t.tile([1, RPC], F32, tag="tsd")
                nc.scalar.activation(out=tsd, in_=psq, func=AF.Sqrt,
                                     scale=1.0 / dim, bias=eps_sb)
                rstd = p1t.tile([1, RPC], F32, tag="rstd")
                nc.vector.reciprocal(rstd, tsd)
                bcast = p1t.tile([128, RPC], F32, tag="bcast")
                nc.gpsimd.partition_broadcast(bcast, rstd[:, :])
                for j in range(n_ch):
                    nc.vector.tensor_mul(dst_sb[:, dst_off + j, :],
                                         stg[:, j, :], bcast)

            # ---- kv latents + k_rope first, gather early ----
            compress(KCH, wckv_sb, kstg, psq_k)
            pkr = p1pk.tile([64, RPC], F32, tag="pkr")
            for kc in range(D // 128):
                nc.tensor.matmul(pkr, wkr_sb[:, kc, :], xt_sb[:, kc, :],
                                 start=(kc == 0), stop=(kc == D // 128 - 1))
            krstg = p1t.tile([64, RPC], F32, tag="krstg")
            nc.scalar.activation(out=krstg, in_=pkr, func=AF.Copy)
            nc.sync.dma_start(out=kro[:, :], in_=krstg)
            nc.vector.tensor_copy(out=latkv_sb[0:64, KCH, :], in_=krstg)
            nc.vector.memset(latkv_sb[64:128, KCH, :], 0.0)
            normalize(kstg, psq_k, KCH, KVR, latkv_sb, 0)
            nc.sync.dma_start(
                out=lat_kv_loc[:, :, :].rearrange("c p r -> p c r"), in_=latkv_sb)
            with tc.tile_critical():
                with nc.semaphore() as csem1:
                    nc.gpsimd.collective_compute(
                        "AllGather", mybir.AluOpType.bypass,
                        replica_groups=[list(range(N_CORES))],
                        ins=[lat_kv_loc[:]], outs=[lat_kv_g[:]],
                    ).then_inc(csem1, 1)
                    nc.gpsimd.wait_ge(csem1, 1)

            # ---- q latents, gather second ----
            compress(QCH, wcq_sb, qstg, psq_q)
            normalize(qstg, psq_q, QCH, QR, latq_sb, 0)
            nc.sync.dma_start(
                out=lat_q_loc[:, :, :].rearrange("c p r -> p c r"), in_=latq_sb)
            with tc.tile_critical():
                with nc.semaphore() as csem2:
                    nc.gpsimd.collective_compute(
                        "AllGather", mybir.AluOpType.bypass,
                        replica_groups=[list(range(N_CORES))],
                        ins=[lat_q_loc[:]], outs=[lat_q_g[:]],
                    ).then_inc(csem2, 1)
                    nc.gpsimd.wait_ge(csem2, 1)

        # ============ phase 2: decompress q/k/v for own heads ============
        caches_cm = tc.tile_pool(name="caches", bufs=1)
        caches = caches_cm.__enter__()
        qn_cache = caches.tile([128, HPC, R], BF, tag="qn_cache")
        qrc0 = caches.tile([64, R], BF, tag="qrc0")
        qrc1 = caches.tile([64, R], BF, tag="qrc1")
        qrc = [qrc0, qrc1]
        kn_cache = caches.tile([128, HPC, R], BF, tag="kn_cache")
        kr_cache = caches.tile([64, R], BF, tag="kr_cache")
        v_cache = caches.tile([128, R // 128, 2 * VD], BF, tag="v_cache")
        cs_sb = caches.tile([32, R], F32, tag="cs_sb")
        sn_sb = caches.tile([32, R], F32, tag="sn_sb")
        nc.sync.dma_start(out=cs_sb, in_=cos32[:, :])
        nc.sync.dma_start(out=sn_sb, in_=sin32[:, :])

        with tc.tile_pool(name="p2w", bufs=1) as p2w, \
             tc.tile_pool(name="p2l", bufs=2) as p2l, \
             tc.tile_pool(name="p2t", bufs=4) as p2t, \
             tc.tile_pool(name="p2p", bufs=4, space="PSUM") as p2p:
            wdqn_sb = p2w.tile([128, QCH, 2 * ND], BF, tag="wdqn")
            nc.sync.dma_start(out=wdqn_sb,
                              in_=wdqn[:, :].rearrange("(kc p) m -> p kc m", p=128))
            wdqr_sb = p2w.tile([128, QCH, 2 * RD], BF, tag="wdqr")
            nc.sync.dma_start(out=wdqr_sb,
                              in_=wdqr[:, :].rearrange("(kc p) m -> p kc m", p=128))
            wdkn_sb = p2w.tile([128, KCH, 2 * ND], BF, tag="wdkn")
            nc.sync.dma_start(out=wdkn_sb,
                              in_=wdkn[:, :].rearrange("(kc p) m -> p kc m", p=128))
            wdv_sb = p2w.tile([128, KCH, 2 * VD], BF, tag="wdv")
            nc.sync.dma_start(out=wdv_sb,
                              in_=wdv[:, :].rearrange("(kc p) m -> p kc m", p=128))

            # ---- pass A: kv-dependent (gated on kv gather) ----
            for rs in range(N_CORES):
                cols = slice(rs * RPC, (rs + 1) * RPC)
                kvlat_sb = p2l.tile([128, KCH, RPC], BF, tag="kvlat")
                nc.sync.dma_start(
                    out=kvlat_sb,
                    in_=lat_kv_g[rs, 0:KCH, :, :].rearrange("c p r -> p c r"))
                krraw_sb = p2l.tile([64, RPC], BF, tag="krraw")
                nc.sync.dma_start(out=krraw_sb, in_=lat_kv_g[rs, KCH, 0:64, :])
                cs, sn = cs_sb[:, cols], sn_sb[:, cols]

                # k_nope^T cache (per head)
                for h in range(HPC):
                    pk = p2p.tile([128, RPC], F32, tag="pdec")
                    for kc in range(KCH):
                        nc.tensor.matmul(pk, wdkn_sb[:, kc, h * ND:(h + 1) * ND],
                                         kvlat_sb[:, kc, :],
                                         start=(kc == 0), stop=(kc == KCH - 1))
                    nc.scalar.activation(out=kn_cache[:, h, cols], in_=pk,
                                         func=AF.Copy)

                # k_rope: shared across heads (split to base-0 tiles first)
                krf1 = p2t.tile([32, RPC], F32, tag="krf1")
                krf2 = p2t.tile([32, RPC], F32, tag="krf2")
                nc.vector.tensor_copy(out=krf1, in_=krraw_sb[0:32, :])
                nc.vector.tensor_copy(out=krf2, in_=krraw_sb[32:64, :])
                t1 = p2t.tile([32, RPC], F32, tag="ropet")
                t2 = p2t.tile([32, RPC], F32, tag="ropet")
                nc.vector.tensor_mul(t1, krf1, cs)
                nc.vector.tensor_mul(t2, krf2, sn)
                nc.vector.tensor_sub(kr_cache[0:32, cols], t1, t2)
                nc.vector.tensor_mul(t1, krf1, sn)
                nc.vector.tensor_mul(t2, krf2, cs)
                nc.vector.tensor_add(kr_cache[32:64, cols], t1, t2)

                # v and k_nope in natural layout (lhsT = kv_lat^T chunk)
                for rc in range(RPC // 128):
                    grow = rs * RPC + rc * 128
                    pv = p2p.tile([128, 2 * VD], F32, tag="pnat")
                    for kc in range(KCH):
                        nc.tensor.matmul(pv, kvlat_sb[:, kc, rc * 128:(rc + 1) * 128],
                                         wdv_sb[:, kc, :],
                                         start=(kc == 0), stop=(kc == KCH - 1))
                    vstg = p2t.tile([128, 2 * VD], F32, tag="vstg")
                    nc.scalar.activation(out=vstg, in_=pv, func=AF.Copy)
                    nc.sync.dma_start(out=vo[grow:grow + 128, :], in_=vstg)
                    nc.vector.tensor_copy(out=v_cache[:, rs * 4 + rc, :], in_=pv)

                    pko = p2p.tile([128, 2 * ND], F32, tag="pnat")
                    for kc in range(KCH):
                        nc.tensor.matmul(pko, kvlat_sb[:, kc, rc * 128:(rc + 1) * 128],
                                         wdkn_sb[:, kc, :],
                                         start=(kc == 0), stop=(kc == KCH - 1))
                    kostg = p2t.tile([128, 2 * ND], F32, tag="vstg")
                    nc.scalar.activation(out=kostg, in_=pko, func=AF.Copy)
                    nc.sync.dma_start(out=kno[grow:grow + 128, :], in_=kostg)

            # ---- pass B: q-dependent (gated on q gather) ----
            for rs in range(N_CORES):
                cols = slice(rs * RPC, (rs + 1) * RPC)
                qlat_sb = p2l.tile([128, QCH, RPC], BF, tag="qlat")
                nc.sync.dma_start(
                    out=qlat_sb,
                    in_=lat_q_g[rs, 0:QCH, :, :].rearrange("c p r -> p c r"))
                cs, sn = cs_sb[:, cols], sn_sb[:, cols]

                for h in range(HPC):
                    pq = p2p.tile([128, RPC], F32, tag="pdec")
                    for kc in range(QCH):
                        nc.tensor.matmul(pq, wdqn_sb[:, kc, h * ND:(h + 1) * ND],
                                         qlat_sb[:, kc, :],
                                         start=(kc == 0), stop=(kc == QCH - 1))
                    nc.scalar.activation(out=qn_cache[:, h, cols], in_=pq,
                                         func=AF.Copy)

                # q_rope^T: [h0x1, h0x2, h1x1, h1x2] then rope per head
                pqr = p2p.tile([128, RPC], F32, tag="pdec")
                for kc in range(QCH):
                    nc.tensor.matmul(pqr, wdqr_sb[:, kc, :], qlat_sb[:, kc, :],
                                     start=(kc == 0), stop=(kc == QCH - 1))
                for h in range(HPC):
                    x1, x2 = pqr[h * 64:h * 64 + 32, :], pqr[h * 64 + 32:h * 64 + 64, :]
                    t1 = p2t.tile([32, RPC], F32, tag="ropet")
                    t2 = p2t.tile([32, RPC], F32, tag="ropet")
                    nc.vector.tensor_mul(t1, x1, cs)
                    nc.vector.tensor_mul(t2, x2, sn)
                    nc.vector.tensor_sub(qrc[h][0:32, cols], t1, t2)
                    nc.vector.tensor_mul(t1, x1, sn)
                    nc.vector.tensor_mul(t2, x2, cs)
                    nc.vector.tensor_add(qrc[h][32:64, cols], t1, t2)

        # ============ phase 3: attention + out projection ============
        # Transposed scores over 512-wide q-groups: scoresT[keys 128, q 512]
        # blocks; stationary operands (kn/kr/v chunks) amortize over N=512.
        # exp -> probsT bf16; denominator via ones-matmul; attn@V gives
        # outT [vd, q] = exactly the lhsT layout out-proj needs.
        NQG = T // 512
        with tc.tile_pool(name="p3w", bufs=1) as p3w, \
             tc.tile_pool(name="p3t", bufs=2) as p3t, \
             tc.tile_pool(name="p3pt", bufs=2) as p3pt, \
             tc.tile_pool(name="p3o", bufs=2) as p3o, \
             tc.tile_pool(name="ps_s", bufs=3, space="PSUM") as ps_s, \
             tc.tile_pool(name="ps_d", bufs=1, space="PSUM") as ps_d, \
             tc.tile_pool(name="ps_o", bufs=2, space="PSUM") as ps_o, \
             tc.tile_pool(name="ps_op", bufs=2, space="PSUM") as ps_op:
            wout_sb = p3w.tile([128, 2, D], BF, tag="wout")
            nc.sync.dma_start(out=wout_sb,
                              in_=wout[:, :].rearrange("(hc p) d -> p hc d", p=128))
            m4_sb = p3w.tile([128, 4, 512], F32, tag="m4")
            nc.sync.dma_start(out=m4_sb,
                              in_=masks4[:, :, :].rearrange("m p q -> p m q"))
            ones_bf = p3w.tile([128, 1], BF, tag="ones_bf")
            nc.vector.memset(ones_bf, 1.0)

            for b in range(B):
                for qg in range(NQG):
                    qcols = slice(b * T + qg * 512, b * T + (qg + 1) * 512)
                    aoqT = p3o.tile([128, HPC, 512], BF, tag="aoqT")
                    njc = 4 * (qg + 1)
                    for h in range(HPC):
                        pT_all = p3pt.tile([128, NQT, 512], BF, tag="pT",
                                           name=f"pT_{b}_{qg}_{h}")
                        for j in range(njc):
                            kcols = slice(b * T + j * 128, b * T + (j + 1) * 128)
                            ps = ps_s.tile([128, 512], F32, tag="ps")
                            nc.tensor.matmul(ps, kn_cache[:, h, kcols],
                                             qn_cache[:, h, qcols],
                                             start=True, stop=False,
                                             skip_group_check=True)
                            nc.tensor.matmul(ps, kr_cache[:, kcols],
                                             qrc[h][:, qcols],
                                             start=False, stop=True,
                                             skip_group_check=True)
                            m = j - 4 * qg
                            if m >= 0:
                                nc.vector.tensor_add(ps, ps, m4_sb[:, m, :])
                            nc.scalar.activation(out=pT_all[:, j, :], in_=ps,
                                                 func=AF.Exp, scale=SCALE)
                        pd = ps_d.tile([1, 512], F32, tag="pd")
                        for j in range(njc):
                            nc.tensor.matmul(pd, ones_bf, pT_all[:, j, :],
                                             start=(j == 0), stop=(j == njc - 1),
                                             skip_group_check=True)
                        po = ps_o.tile([128, 512], F32, tag="po")
                        for j in range(njc):
                            nc.tensor.matmul(po, v_cache[:, b * NQT + j,
                                                         h * VD:(h + 1) * VD],
                                             pT_all[:, j, :],
                                             start=(j == 0), stop=(j == njc - 1),
                                             skip_group_check=True)
                        rdf = p3t.tile([1, 512], F32, tag="rdf")
                        nc.vector.reciprocal(rdf, pd)
                        bcast = p3t.tile([128, 512], F32, tag="bcast")
                        nc.gpsimd.partition_broadcast(bcast, rdf[:, :])
                        nc.vector.tensor_mul(aoqT[:, h, :], po, bcast)

                    # out projection for these 512 rows
                    for rc in range(4):
                        rows = slice(b * T + qg * 512 + rc * 128,
                                     b * T + qg * 512 + (rc + 1) * 128)
                        ostg = p3o.tile([128, D], F32, tag="ostg")
                        for nc4 in range(D // 512):
                            pop = ps_op.tile([128, 512], F32, tag="pop")
                            for h in range(HPC):
                                nc.tensor.matmul(
                                    pop, aoqT[:, h, rc * 128:(rc + 1) * 128],
                                    wout_sb[:, h, nc4 * 512:(nc4 + 1) * 512],
                                    start=(h == 0), stop=(h == HPC - 1))
                            if nc4 % 2 == 0:
                                nc.scalar.activation(
                                    out=ostg[:, nc4 * 512:(nc4 + 1) * 512],
                                    in_=pop, func=AF.Copy)
                            else:
                                nc.vector.tensor_copy(
                                    out=ostg[:, nc4 * 512:(nc4 + 1) * 512], in_=pop)
                        nc.sync.dma_start(out=outp[rows, :], in_=ostg)

        caches_cm.__exit__(None, None, None)

    nc.finalize()
    return nc


_PROGRAM = None


def _get_program():
    global _PROGRAM
    if _PROGRAM is None:
        _PROGRAM = build_program()
    return _PROGRAM


def run_device(in_maps, trace=False):
    _register_ntff_hook()
    from concourse.bass_utils import run_bass_kernel_spmd
    nc = _get_program()
    res = run_bass_kernel_spmd(nc, in_maps, list(range(N_CORES)), trace=trace)
    return res


def assemble(results):
    """Host-side: sum out partials, assemble k and v_t."""
    out = np.zeros((R, D), np.float32)
    for c in range(N_CORES):
        out += results[c]["outp"]
    out = out.reshape(B, T, D)

    k = np.empty((B, H, T, ND + RD), np.float32)
    v_t = np.empty((B, H, T, VD), np.float32)
    for c in range(N_CORES):
        kno = results[c]["kno"]          # [R, 2*ND]
        vo = results[c]["vo"]            # [R, 2*VD]
        for j in range(HPC):
            h = HPC * c + j
            k[:, h, :, :ND] = kno[:, j * ND:(j + 1) * ND].reshape(B, T, ND)
            v_t[:, h] = vo[:, j * VD:(j + 1) * VD].reshape(B, T, VD)

    # k rope part: gather raw slabs, undo deinterleave, rope on host (fp32)
    kr_all = np.empty((R, RD), np.float32)
    for c in range(N_CORES):
        kr_all[c * RPC:(c + 1) * RPC] = results[c]["kro"].T
    perm64 = _deint(RD)
    kr_orig = np.empty_like(kr_all)
    kr_orig[:, perm64] = kr_all          # invert permutation
    inv_freq = 1.0 / (10000.0 ** (np.arange(0, RD, 2, dtype=np.float32) / RD))
    t_of_row = np.tile(np.arange(T, dtype=np.float32), B)
    ang = t_of_row[:, None] * inv_freq[None, :]
    cos, sin = np.cos(ang), np.sin(ang)
    x1, x2 = kr_orig[:, 0::2], kr_orig[:, 1::2]
    o = np.empty_like(kr_orig)
    o[:, 0::2] = x1 * cos - x2 * sin
    o[:, 1::2] = x1 * sin + x2 * cos
    k[:, :, :, ND:] = o.reshape(B, T, RD)[:, None, :, :]
    return out, k, v_t


def kernel(**inputs):
    in_maps = host_prepare(**inputs)
    res = run_device(in_maps)
    return assemble(res.results)


# revision 30
# speedup vs baseline: 1.2137x; 1.0317x over previous
"""MLA (multi-head latent attention) forward on 8 Trainium2 NeuronCores.

Sharding: token-sharded compress (low-rank latents) + AllGather of latents +
head-sharded attention (2 heads/core) + head-sliced out-projection partials
summed on host.

All device matmuls run in bf16 with fp32 PSUM accumulation. Activations are
held transposed ([feature, token]) so every matmul contracts along partitions
with zero on-device transposes of activations (probs transposes for attn@V go
through the PE).

Self-contained: hardcodes the problem shapes from the reference
(B=2, T=2048, D=2048, H=16, ND=128, RD=64, VD=128, QR=1536, KVR=512).
"""
import math
import sys
import types

import numpy as np
import ml_dtypes

BF16 = ml_dtypes.bfloat16

# problem shapes
B, T, D = 2, 2048, 2048
H, ND, RD, VD = 16, 128, 64, 128
QR, KVR = 1536, 512
EPS = 1e-6
N_CORES = 8
HPC = H // N_CORES            # heads per core = 2
R = B * T                     # 4096 global rows
RPC = R // N_CORES            # 512 rows per core slab
NQT = T // 128                # 16 q-tiles per batch
SCALE = 1.0 / math.sqrt(ND + RD)
NEG = -30000.0

# latent chunk layout: 12 q chunks, 4 kv chunks, 1 krope chunk (64 rows used)
QCH = QR // 128               # 12
KCH = KVR // 128              # 4
LCH = QCH + KCH + 1           # 17


def _register_ntff_hook():
    try:
        import antenv
    except ImportError:
        return
    if "antenv.axon_hooks" in sys.modules:
        return
    mod = types.ModuleType("antenv.axon_hooks")
    mod._hook = None
    mod.set_axon_ntff_profile_hook = lambda h: setattr(mod, "_hook", h)
    mod.get_axon_ntff_profile_hook = lambda: mod._hook
    sys.modules["antenv.axon_hooks"] = mod
    antenv.axon_hooks = mod
    try:
        from trn_agent_boot.trn_boot import _ntff_profile_via_ctypes
        mod.set_axon_ntff_profile_hook(
            _ntff_profile_via_ctypes("/opt/axon/libaxon_pjrt.so"))
    except Exception:
        pass


def _deint(n):
    """deinterleave permutation for rope dims: [0,2,...,n-2, 1,3,...,n-1]"""
    return np.concatenate([np.arange(0, n, 2), np.arange(1, n, 2)])


def host_prepare(x, w_cq, g_q, w_ckv, g_kv, w_dq_nope, w_dq_rope,
                 w_dk_nope, w_dv, w_k_rope, w_out):
    """Build per-core input maps (numpy, bf16 for matmul operands)."""
    x = np.asarray(x, np.float32)
    xf = x.reshape(R, D)
    perm64 = _deint(RD)

    # fold rmsnorm gains into decompress weights
    wdqn_f = np.asarray(w_dq_nope, np.float32) * np.asarray(g_q, np.float32)[:, None]
    wdqr_f = np.asarray(w_dq_rope, np.float32) * np.asarray(g_q, np.float32)[:, None]
    wdkn_f = np.asarray(w_dk_nope, np.float32) * np.asarray(g_kv, np.float32)[:, None]
    wdv_f = np.asarray(w_dv, np.float32) * np.asarray(g_kv, np.float32)[:, None]

    wcq = np.asarray(w_cq, np.float32).astype(BF16)
    wckv = np.asarray(w_ckv, np.float32).astype(BF16)
    wkr = np.asarray(w_k_rope, np.float32)[:, perm64].astype(BF16)

    # rope tables, deinterleaved freq order, indexed by global row (b*T + t)
    inv_freq = 1.0 / (10000.0 ** (np.arange(0, RD, 2, dtype=np.float32) / RD))  # [32]
    t_of_row = np.tile(np.arange(T, dtype=np.float32), B)                       # [R]
    ang = inv_freq[:, None] * t_of_row[None, :]                                 # [32, R]
    cos32 = np.cos(ang).astype(np.float32)
    sin32 = np.sin(ang).astype(np.float32)

    # transposed causal masks for 512-wide q-groups: for relative key chunk m
    # (0..3), maskT[m][kr, qc] = 0 if (m*128 + kr) <= qc else NEG
    kr = np.arange(128)[:, None]
    qc = np.arange(512)[None, :]
    masks4 = np.stack([np.where(m * 128 + kr <= qc, 0.0, NEG)
                       for m in range(4)]).astype(np.float32)

    in_maps = []
    for c in range(N_CORES):
        h0, h1 = 2 * c, 2 * c + 1
        # w_dq_rope columns for head h: [h*RD, (h+1)*RD); per-head [even32, odd32]
        qr_cols = np.concatenate([h0 * RD + perm64, h1 * RD + perm64])
        nope_cols = np.concatenate(
            [np.arange(h0 * ND, (h0 + 1) * ND), np.arange(h1 * ND, (h1 + 1) * ND)])
        v_cols = np.concatenate(
            [np.arange(h0 * VD, (h0 + 1) * VD), np.arange(h1 * VD, (h1 + 1) * VD)])
        in_maps.append({
            "xt": np.ascontiguousarray(xf[c * RPC:(c + 1) * RPC].T).astype(BF16),
            "wcq": wcq,
            "wckv": wckv,
            "wkr": wkr,
            "wdqn": wdqn_f[:, nope_cols].astype(BF16),
            "wdqr": wdqr_f[:, qr_cols].astype(BF16),
            "wdkn": wdkn_f[:, nope_cols].astype(BF16),
            "wdv": wdv_f[:, v_cols].astype(BF16),
            "wout": np.ascontiguousarray(
                np.asarray(w_out, np.float32)[v_cols, :]).astype(BF16),
            "cos32": cos32,
            "sin32": sin32,
            "masks4": masks4,
        })
    return in_maps


def build_program():
    import concourse.bass as bass
    import concourse.tile as tile
    from concourse import bacc, mybir

    dt = mybir.dt
    F32, BF = dt.float32, dt.bfloat16
    AF = mybir.ActivationFunctionType

    nc = bacc.Bacc(None, target_bir_lowering=False)

    # ---- I/O ----
    xt = nc.declare_dram_parameter("xt", [D, RPC], BF, isOutput=False)
    wcq = nc.declare_dram_parameter("wcq", [D, QR], BF, isOutput=False)
    wckv = nc.declare_dram_parameter("wckv", [D, KVR], BF, isOutput=False)
    wkr = nc.declare_dram_parameter("wkr", [D, RD], BF, isOutput=False)
    wdqn = nc.declare_dram_parameter("wdqn", [QR, 2 * ND], BF, isOutput=False)
    wdqr = nc.declare_dram_parameter("wdqr", [QR, 2 * RD], BF, isOutput=False)
    wdkn = nc.declare_dram_parameter("wdkn", [KVR, 2 * ND], BF, isOutput=False)
    wdv = nc.declare_dram_parameter("wdv", [KVR, 2 * VD], BF, isOutput=False)
    wout = nc.declare_dram_parameter("wout", [2 * VD, D], BF, isOutput=False)
    cos32 = nc.declare_dram_parameter("cos32", [RD // 2, R], F32, isOutput=False)
    sin32 = nc.declare_dram_parameter("sin32", [RD // 2, R], F32, isOutput=False)
    masks4 = nc.declare_dram_parameter("masks4", [4, 128, 512], F32, isOutput=False)

    kno = nc.declare_dram_parameter("kno", [R, 2 * ND], F32, isOutput=True)
    vo = nc.declare_dram_parameter("vo", [R, 2 * VD], F32, isOutput=True)
    kro = nc.declare_dram_parameter("kro", [RD, RPC], F32, isOutput=True)
    outp = nc.declare_dram_parameter("outp", [R, D], F32, isOutput=True)

    lat_kv_loc = nc.dram_tensor("lat_kv_loc", [KCH + 1, 128, RPC], BF)
    lat_q_loc = nc.dram_tensor("lat_q_loc", [QCH, 128, RPC], BF)
    lat_kv_g = nc.dram_tensor("lat_kv_g", [N_CORES, KCH + 1, 128, RPC], BF,
                              addr_space="Shared")
    lat_q_g = nc.dram_tensor("lat_q_g", [N_CORES, QCH, 128, RPC], BF,
                             addr_space="Shared")

    with tile.TileContext(nc) as tc:
        # ================= phase 1: compress own slab =================
        with tc.tile_pool(name="p1w", bufs=1) as p1w, \
             tc.tile_pool(name="p1s", bufs=1) as p1s, \
             tc.tile_pool(name="p1t", bufs=3) as p1t, \
             tc.tile_pool(name="p1p", bufs=2, space="PSUM") as p1p, \
             tc.tile_pool(name="p1pk", bufs=1, space="PSUM") as p1pk, \
             tc.tile_pool(name="p1q", bufs=1, space="PSUM") as p1q:
            xt_sb = p1w.tile([128, D // 128, RPC], BF, tag="xt")
            nc.sync.dma_start(out=xt_sb,
                              in_=xt[:, :].rearrange("(kc p) r -> p kc r", p=128))
            wckv_sb = p1w.tile([128, D // 128, KVR], BF, tag="wckv")
            nc.sync.dma_start(out=wckv_sb,
                              in_=wckv[:, :].rearrange("(kc p) m -> p kc m", p=128))
            wkr_sb = p1w.tile([128, D // 128, RD], BF, tag="wkr")
            nc.sync.dma_start(out=wkr_sb,
                              in_=wkr[:, :].rearrange("(kc p) m -> p kc m", p=128))
            wcq_sb = p1w.tile([128, D // 128, QR], BF, tag="wcq")
            nc.sync.dma_start(out=wcq_sb,
                              in_=wcq[:, :].rearrange("(kc p) m -> p kc m", p=128))
            ones128 = p1w.tile([128, 1], BF, tag="ones128")
            nc.vector.memset(ones128, 1.0)
            eps_sb = p1w.tile([1, 1], F32, tag="eps_sb")
            nc.vector.memset(eps_sb, EPS)

            qstg = p1s.tile([128, QCH, RPC], F32, tag="qstg")
            kstg = p1s.tile([128, KCH, RPC], F32, tag="kstg")
            latq_sb = p1s.tile([128, QCH, RPC], BF, tag="latq_sb")
            latkv_sb = p1s.tile([128, KCH + 1, RPC], BF, tag="latkv_sb")

            psq_q = p1q.tile([1, RPC], F32, tag="psq_q")
            psq_k = p1q.tile([1, RPC], F32, tag="psq_k")

            def compress(n_ch, w_sb, stg, psq):
                for j in range(n_ch):
                    pc = p1p.tile([128, RPC], F32, tag="pc")
                    for kc in range(D // 128):
                        nc.tensor.matmul(pc, w_sb[:, kc, j * 128:(j + 1) * 128],
                                         xt_sb[:, kc, :],
                                         start=(kc == 0), stop=(kc == D // 128 - 1))
                    nc.scalar.activation(out=stg[:, j, :], in_=pc, func=AF.Copy)
                    sq = p1t.tile([128, RPC], BF, tag="sq")
                    nc.scalar.activation(out=sq, in_=pc, func=AF.Square)
                    nc.tensor.matmul(psq, ones128, sq,
                                     start=(j == 0), stop=(j == n_ch - 1),
                                     skip_group_check=True)

            def normalize(stg, psq, n_ch, dim, dst_sb, dst_off):
                tsd = p1t.tile([1, RPC], F32, tag="tsd")
                nc.scalar.activation(out=tsd, in_=psq, func=AF.Sqrt,
                                     scale=1.0 / dim, bias=eps_sb)
                rstd = p1t.tile([1, RPC], F32, tag="rstd")
                nc.vector.reciprocal(rstd, tsd)
                bcast = p1t.tile([128, RPC], F32, tag="bcast")
                nc.gpsimd.partition_broadcast(bcast, rstd[:, :])
                for j in range(n_ch):
                    nc.vector.tensor_mul(dst_sb[:, dst_off + j, :],
                                         stg[:, j, :], bcast)

            # ---- kv latents + k_rope first, gather early ----
            compress(KCH, wckv_sb, kstg, psq_k)
            pkr = p1pk.tile([64, RPC], F32, tag="pkr")
            for kc in range(D // 128):
                nc.tensor.matmul(pkr, wkr_sb[:, kc, :], xt_sb[:, kc, :],
                                 start=(kc == 0), stop=(kc == D // 128 - 1))
            krstg = p1t.tile([64, RPC], F32, tag="krstg")
            nc.scalar.activation(out=krstg, in_=pkr, func=AF.Copy)
            nc.sync.dma_start(out=kro[:, :], in_=krstg)
            nc.vector.tensor_copy(out=latkv_sb[0:64, KCH, :], in_=krstg)
            nc.vector.memset(latkv_sb[64:128, KCH, :], 0.0)
            normalize(kstg, psq_k, KCH, KVR, latkv_sb, 0)
            nc.sync.dma_start(
                out=lat_kv_loc[:, :, :].rearrange("c p r -> p c r"), in_=latkv_sb)
            with tc.tile_critical():
                with nc.semaphore() as csem1:
                    nc.gpsimd.collective_compute(
                        "AllGather", mybir.AluOpType.bypass,
                        replica_groups=[list(range(N_CORES))],
                        ins=[lat_kv_loc[:]], outs=[lat_kv_g[:]],
                    ).then_inc(csem1, 1)
                    nc.gpsimd.wait_ge(csem1, 1)

            # ---- q latents, gather second ----
            compress(QCH, wcq_sb, qstg, psq_q)
            normalize(qstg, psq_q, QCH, QR, latq_sb, 0)
            nc.sync.dma_start(
                out=lat_q_loc[:, :, :].rearrange("c p r -> p c r"), in_=latq_sb)
            with tc.tile_critical():
                with nc.semaphore() as csem2:
                    nc.gpsimd.collective_compute(
                        "AllGather", mybir.AluOpType.bypass,
                        replica_groups=[list(range(N_CORES))],
                        ins=[lat_q_loc[:]], outs=[lat_q_g[:]],
                    ).then_inc(csem2, 1)
                    nc.gpsimd.wait_ge(csem2, 1)

        # ============ phase 2: decompress q/k/v for own heads ============
        caches_cm = tc.tile_pool(name="caches", bufs=1)
        caches = caches_cm.__enter__()
        qn_cache = caches.tile([128, HPC, R], BF, tag="qn_cache")
        qrc0 = caches.tile([64, R], BF, tag="qrc0")
        qrc1 = caches.tile([64, R], BF, tag="qrc1")
        qrc = [qrc0, qrc1]
        kn_cache = caches.tile([128, HPC, R], BF, tag="kn_cache")
        kr_cache = caches.tile([64, R], BF, tag="kr_cache")
        v_cache = caches.tile([128, R // 128, 2 * VD], BF, tag="v_cache")
        cs_sb = caches.tile([32, R], F32, tag="cs_sb")
        sn_sb = caches.tile([32, R], F32, tag="sn_sb")
        nc.sync.dma_start(out=cs_sb, in_=cos32[:, :])
        nc.sync.dma_start(out=sn_sb, in_=sin32[:, :])

        with tc.tile_pool(name="p2w", bufs=1) as p2w, \
             tc.tile_pool(name="p2l", bufs=2) as p2l, \
             tc.tile_pool(name="p2t", bufs=4) as p2t, \
             tc.tile_pool(name="p2p", bufs=4, space="PSUM") as p2p:
            wdqn_sb = p2w.tile([128, QCH, 2 * ND], BF, tag="wdqn")
            nc.sync.dma_start(out=wdqn_sb,
                              in_=wdqn[:, :].rearrange("(kc p) m -> p kc m", p=128))
            wdqr_sb = p2w.tile([128, QCH, 2 * RD], BF, tag="wdqr")
            nc.sync.dma_start(out=wdqr_sb,
                              in_=wdqr[:, :].rearrange("(kc p) m -> p kc m", p=128))
            wdkn_sb = p2w.tile([128, KCH, 2 * ND], BF, tag="wdkn")
            nc.sync.dma_start(out=wdkn_sb,
                              in_=wdkn[:, :].rearrange("(kc p) m -> p kc m", p=128))
            wdv_sb = p2w.tile([128, KCH, 2 * VD], BF, tag="wdv")
            nc.sync.dma_start(out=wdv_sb,
                              in_=wdv[:, :].rearrange("(kc p) m -> p kc m", p=128))

            # ---- pass A: kv-dependent (gated on kv gather) ----
            for rs in range(N_CORES):
                cols = slice(rs * RPC, (rs + 1) * RPC)
                kvlat_sb = p2l.tile([128, KCH, RPC], BF, tag="kvlat")
                nc.sync.dma_start(
                    out=kvlat_sb,
                    in_=lat_kv_g[rs, 0:KCH, :, :].rearrange("c p r -> p c r"))
                krraw_sb = p2l.tile([64, RPC], BF, tag="krraw")
                nc.sync.dma_start(out=krraw_sb, in_=lat_kv_g[rs, KCH, 0:64, :])
                cs, sn = cs_sb[:, cols], sn_sb[:, cols]

                # k_nope^T cache (per head)
                for h in range(HPC):
                    pk = p2p.tile([128, RPC], F32, tag="pdec")
                    for kc in range(KCH):
                        nc.tensor.matmul(pk, wdkn_sb[:, kc, h * ND:(h + 1) * ND],
                                         kvlat_sb[:, kc, :],
                                         start=(kc == 0), stop=(kc == KCH - 1))
                    nc.scalar.activation(out=kn_cache[:, h, cols], in_=pk,
                                         func=AF.Copy)

                # k_rope: shared across heads (split to base-0 tiles first)
                krf1 = p2t.tile([32, RPC], F32, tag="krf1")
                krf2 = p2t.tile([32, RPC], F32, tag="krf2")
                nc.vector.tensor_copy(out=krf1, in_=krraw_sb[0:32, :])
                nc.vector.tensor_copy(out=krf2, in_=krraw_sb[32:64, :])
                t1 = p2t.tile([32, RPC], F32, tag="ropet")
                t2 = p2t.tile([32, RPC], F32, tag="ropet")
                nc.vector.tensor_mul(t1, krf1, cs)
                nc.vector.tensor_mul(t2, krf2, sn)
                nc.vector.tensor_sub(kr_cache[0:32, cols], t1, t2)
                nc.vector.tensor_mul(t1, krf1, sn)
                nc.vector.tensor_mul(t2, krf2, cs)
                nc.vector.tensor_add(kr_cache[32:64, cols], t1, t2)

                # v and k_nope in natural layout (lhsT = kv_lat^T chunk)
                for rc in range(RPC // 128):
                    grow = rs * RPC + rc * 128
                    pv = p2p.tile([128, 2 * VD], F32, tag="pnat")
                    for kc in range(KCH):
                        nc.tensor.matmul(pv, kvlat_sb[:, kc, rc * 128:(rc + 1) * 128],
                                         wdv_sb[:, kc, :],
                                         start=(kc == 0), stop=(kc == KCH - 1))
                    vstg = p2t.tile([128, 2 * VD], F32, tag="vstg")
                    nc.scalar.activation(out=vstg, in_=pv, func=AF.Copy)
                    nc.sync.dma_start(out=vo[grow:grow + 128, :], in_=vstg)
                    nc.vector.tensor_copy(out=v_cache[:, rs * 4 + rc, :], in_=pv)

                    pko = p2p.tile([128, 2 * ND], F32, tag="pnat")
                    for kc in range(KCH):
                        nc.tensor.matmul(pko, kvlat_sb[:, kc, rc * 128:(rc + 1) * 128],
                                         wdkn_sb[:, kc, :],
                                         start=(kc == 0), stop=(kc == KCH - 1))
                    kostg = p2t.tile([128, 2 * ND], F32, tag="vstg")
                    nc.scalar.activation(out=kostg, in_=pko, func=AF.Copy)
                    nc.sync.dma_start(out=kno[grow:grow + 128, :], in_=kostg)

            # ---- pass B: q-dependent (gated on q gather) ----
            for rs in range(N_CORES):
                cols = slice(rs * RPC, (rs + 1) * RPC)
                qlat_sb = p2l.tile([128, QCH, RPC], BF, tag="qlat")
                nc.sync.dma_start(
                    out=qlat_sb,
                    in_=lat_q_g[rs, 0:QCH, :, :].rearrange("c p r -> p c r"))
                cs, sn = cs_sb[:, cols], sn_sb[:, cols]

                for h in range(HPC):
                    pq = p2p.tile([128, RPC], F32, tag="pdec")
                    for kc in range(QCH):
                        nc.tensor.matmul(pq, wdqn_sb[:, kc, h * ND:(h + 1) * ND],
                                         qlat_sb[:, kc, :],
                                         start=(kc == 0), stop=(kc == QCH - 1))
                    nc.scalar.activation(out=qn_cache[:, h, cols], in_=pq,
                                         func=AF.Copy)

                # q_rope^T: [h0x1, h0x2, h1x1, h1x2] then rope per head
                pqr = p2p.tile([128, RPC], F32, tag="pdec")
                for kc in range(QCH):
                    nc.tensor.matmul(pqr, wdqr_sb[:, kc, :], qlat_sb[:, kc, :],
                                     start=(kc == 0), stop=(kc == QCH - 1))
                for h in range(HPC):
                    x1, x2 = pqr[h * 64:h * 64 + 32, :], pqr[h * 64 + 32:h * 64 + 64, :]
                    t1 = p2t.tile([32, RPC], F32, tag="ropet")
                    t2 = p2t.tile([32, RPC], F32, tag="ropet")
                    nc.vector.tensor_mul(t1, x1, cs)
                    nc.vector.tensor_mul(t2, x2, sn)
                    nc.vector.tensor_sub(qrc[h][0:32, cols], t1, t2)
                    nc.vector.tensor_mul(t1, x1, sn)
                    nc.vector.tensor_mul(t2, x2, cs)
                    nc.vector.tensor_add(qrc[h][32:64, cols], t1, t2)

        # ============ phase 3: attention + out projection ============
        # Transposed scores over 512-wide q-groups: scoresT[keys 128, q 512]
        # blocks; stationary operands (kn/kr/v chunks) amortize over N=512.
        # exp -> probsT bf16; denominator via ones-matmul; attn@V gives
        # outT [vd, q] = exactly the lhsT layout out-proj needs.
        NQG = T // 512
        with tc.tile_pool(name="p3w", bufs=1) as p3w, \
             tc.tile_pool(name="p3t", bufs=2) as p3t, \
             tc.tile_pool(name="p3pt", bufs=2) as p3pt, \
             tc.tile_pool(name="p3o", bufs=2) as p3o, \
             tc.tile_pool(name="ps_s", bufs=3, space="PSUM") as ps_s, \
             tc.tile_pool(name="ps_d", bufs=1, space="PSUM") as ps_d, \
             tc.tile_pool(name="ps_o", bufs=2, space="PSUM") as ps_o, \
             tc.tile_pool(name="ps_op", bufs=2, space="PSUM") as ps_op:
            wout_sb = p3w.tile([128, 2, D], BF, tag="wout")
            nc.sync.dma_start(out=wout_sb,
                              in_=wout[:, :].rearrange("(hc p) d -> p hc d", p=128))
            m4_sb = p3w.tile([128, 4, 512], F32, tag="m4")
            nc.sync.dma_start(out=m4_sb,
                              in_=masks4[:, :, :].rearrange("m p q -> p m q"))
            ones_bf = p3w.tile([128, 1], BF, tag="ones_bf")
            nc.vector.memset(ones_bf, 1.0)

            for b in range(B):
                for qg in range(NQG):
                    qcols = slice(b * T + qg * 512, b * T + (qg + 1) * 512)
                    aoqT = p3o.tile([128, HPC, 512], BF, tag="aoqT")
                    njc = 4 * (qg + 1)
                    for h in range(HPC):
                        pT_all = p3pt.tile([128, NQT, 512], BF, tag="pT",
                                           name=f"pT_{b}_{qg}_{h}")
                        for j in range(njc):
                            kcols = slice(b * T + j * 128, b * T + (j + 1) * 128)
                            ps = ps_s.tile([128, 512], F32, tag="ps")
                            nc.tensor.matmul(ps, kn_cache[:, h, kcols],
                                             qn_cache[:, h, qcols],
                                             start=True, stop=False,
                                             skip_group_check=True)
                            nc.tensor.matmul(ps, kr_cache[:, kcols],
                                             qrc[h][:, qcols],
                                             start=False, stop=True,
                                             skip_group_check=True)
                            m = j - 4 * qg
                            if m >= 0:
                                nc.vector.tensor_add(ps, ps, m4_sb[:, m, :])
                            nc.scalar.activation(out=pT_all[:, j, :], in_=ps,
                                                 func=AF.Exp, scale=SCALE)
                        pd = ps_d.tile([1, 512], F32, tag="pd")
                        for j in range(njc):
                            nc.tensor.matmul(pd, ones_bf, pT_all[:, j, :],
                                             start=(j == 0), stop=(j == njc - 1),
                                             skip_group_check=True)
                        po = ps_o.tile([128, 512], F32, tag="po")
                        for j in range(njc):
                            nc.tensor.matmul(po, v_cache[:, b * NQT + j,
                                                         h * VD:(h + 1) * VD],
                                             pT_all[:, j, :],
                                             start=(j == 0), stop=(j == njc - 1),
                                             skip_group_check=True)
                        rdf = p3t.tile([1, 512], F32, tag="rdf")
                        nc.vector.reciprocal(rdf, pd)
                        bcast = p3t.tile([128, 512], F32, tag="bcast")
                        nc.gpsimd.partition_broadcast(bcast, rdf[:, :])
                        nc.vector.tensor_mul(aoqT[:, h, :], po, bcast)

                    # out projection for these 512 rows
                    for rc in range(4):
                        rows = slice(b * T + qg * 512 + rc * 128,
                                     b * T + qg * 512 + (rc + 1) * 128)
                        ostg = p3o.tile([128, D], F32, tag="ostg")
                        for nc4 in range(D // 512):
                            pop = ps_op.tile([128, 512], F32, tag="pop")
                            for h in range(HPC):
                                nc.tensor.matmul(
                                    pop, aoqT[:, h, rc * 128:(rc + 1) * 128],
                                    wout_sb[:, h, nc4 * 512:(nc4 + 1) * 512],
                                    start=(h == 0), stop=(h == HPC - 1))
                            if nc4 % 2 == 0:
                                nc.scalar.activation(
                                    out=ostg[:, nc4 * 512:(nc4 + 1) * 512],
                                    in_=pop, func=AF.Copy)
                            else:
                                nc.vector.tensor_copy(
                                    out=ostg[:, nc4 * 512:(nc4 + 1) * 512], in_=pop)
                        nc.sync.dma_start(out=outp[rows, :], in_=ostg)

        caches_cm.__exit__(None, None, None)

    nc.finalize()
    return nc


_PROGRAM = None


def _get_program():
    global _PROGRAM
    if _PROGRAM is None:
        _PROGRAM = build_program()
    return _PROGRAM


def run_device(in_maps, trace=False):
    _register_ntff_hook()
    from concourse.bass_utils import run_bass_kernel_spmd
    nc = _get_program()
    res = run_bass_kernel_spmd(nc, in_maps, list(range(N_CORES)), trace=trace)
    return res


def assemble(results):
    """Host-side: sum out partials, assemble k and v_t."""
    out = np.zeros((R, D), np.float32)
    for c in range(N_CORES):
        out += results[c]["outp"]
    out = out.reshape(B, T, D)

    k = np.empty((B, H, T, ND + RD), np.float32)
    v_t = np.empty((B, H, T, VD), np.float32)
    for c in range(N_CORES):
        kno = results[c]["kno"]          # [R, 2*ND]
        vo = results[c]["vo"]            # [R, 2*VD]
        for j in range(HPC):
            h = HPC * c + j
            k[:, h, :, :ND] = kno[:, j * ND:(j + 1) * ND].reshape(B, T, ND)
            v_t[:, h] = vo[:, j * VD:(j + 1) * VD].reshape(B, T, VD)

    # k rope part: gather raw slabs, undo deinterleave, rope on host (fp32)
    kr_all = np.empty((R, RD), np.float32)
    for c in range(N_CORES):
        kr_all[c * RPC:(c + 1) * RPC] = results[c]["kro"].T
    perm64 = _deint(RD)
    kr_orig = np.empty_like(kr_all)
    kr_orig[:, perm64] = kr_all          # invert permutation
    inv_freq = 1.0 / (10000.0 ** (np.arange(0, RD, 2, dtype=np.float32) / RD))
    t_of_row = np.tile(np.arange(T, dtype=np.float32), B)
    ang = t_of_row[:, None] * inv_freq[None, :]
    cos, sin = np.cos(ang), np.sin(ang)
    x1, x2 = kr_orig[:, 0::2], kr_orig[:, 1::2]
    o = np.empty_like(kr_orig)
    o[:, 0::2] = x1 * cos - x2 * sin
    o[:, 1::2] = x1 * sin + x2 * cos
    k[:, :, :, ND:] = o.reshape(B, T, RD)[:, None, :, :]
    return out, k, v_t


def kernel(**inputs):
    in_maps = host_prepare(**inputs)
    res = run_device(in_maps)
    return assemble(res.results)
